# revision 1
# baseline (speedup 1.0000x reference)
"""Multi-head causal attention (B=2, T=2048, E=1024, H=16, D=64) on 8 trn2 cores.

Sharding: core c -> batch b = c // 4, head-group hg = c % 4 (4 heads each).
Per-core: QKV projections for its 4 heads, causal flash attention in
transposed-score layout (S^T[k,q]; softmax denominator folded into a
ones-augmented V matmul), row-parallel output projection producing a partial
[T, E] output. Host sums the 4 partials per batch and adds the bias.
"""
import sys
from contextlib import ExitStack

sys.path.insert(0, "/opt/trn_rl_repo")

import numpy as np

import concourse.bass as bass
import concourse.tile as tile
from concourse import bacc, mybir
from concourse.bass_utils import run_bass_kernel_spmd

F32 = mybir.dt.float32
F32R = mybir.dt.float32r
EXP = mybir.ActivationFunctionType.Exp

B, T, E, H = 2, 2048, 1024, 16
D = E // H              # 64
N_CORES = 8
GH = 4                  # heads per core
GE = GH * D             # 256 per-core projection width
SCALE = float(D) ** -0.5

TCH = 512               # projection t-chunk
NTCH = T // TCH         # 4
KC = 8                  # contraction chunks of 128 over E
QB = 512                # attention q-block
NQB = T // QB           # 4
KB = 128                # attention k-block


DEFAULT_OPTS = dict(
    interleave=False,    # head-interleaved emission (PE row-group packing) -- off: modeled slower
    s_bufs=2,            # S psum slots ([128,1024] = 2 banks each)
    y_in_s=False,
    o_bufs=3,
    p_bufs=6,
    x_bufs=10,
    v_bufs=3,
    y_split=True,        # Y psum as two single-bank [128,512] tiles
    y_bufs=1,
    y_last_in_s=True,    # final q-block Y tiles borrow the idle S slots
    recip_fast=False,    # approx recip (custom DVE) produced garbage on HW -- keep exact
    norm_splits_last=4,  # split the last q-block's normalize per q-tile
    l_bufs=6,
    on_bufs=6,
    yst_bufs=4,          # more Y staging slots pipeline the out-projection tail
)


def build_program(opts=None):
    o = dict(DEFAULT_OPTS)
    if opts:
        o.update(opts)
    nc = bacc.Bacc("TRN2", target_bir_lowering=False, debug=False, num_devices=N_CORES)

    xt_d = nc.dram_tensor("xt", [E, T], F32, kind="ExternalInput").ap()
    wqt_d = nc.dram_tensor("wqt", [E, GE], F32, kind="ExternalInput").ap()
    wkt_d = nc.dram_tensor("wkt", [E, GE], F32, kind="ExternalInput").ap()
    wvt_d = nc.dram_tensor("wvt", [E, GE], F32, kind="ExternalInput").ap()
    wpt_d = nc.dram_tensor("wpt", [GE, E], F32, kind="ExternalInput").ap()
    tri_d = nc.dram_tensor("tri", [KB, KB], F32, kind="ExternalInput").ap()
    ones_d = nc.dram_tensor("ones", [128, (T // KB) * GH], F32, kind="ExternalInput").ap()
    y_d = nc.dram_tensor("y", [T, E], F32, kind="ExternalOutput").ap()

    with tile.TileContext(nc) as tc:
        with tc.tile_pool(name="weights", bufs=1) as wpool, \
             tc.tile_pool(name="qk", bufs=1) as qkpool, \
             tc.tile_pool(name="vsb", bufs=1) as vpool, \
             tc.tile_pool(name="xin", bufs=o["x_bufs"]) as xpool, \
             tc.tile_pool(name="ptile", bufs=o["p_bufs"]) as ppool, \
             tc.tile_pool(name="osb", bufs=3) as opool, \
             tc.tile_pool(name="lbc", bufs=o.get("l_bufs", 3)) as lpool, \
             tc.tile_pool(name="onorm", bufs=o.get("on_bufs", 4)) as onpool, \
             tc.tile_pool(name="ystage", bufs=o.get("yst_bufs", 2)) as ypool:

            # --- weights / mask tiles (DMAs emitted inside phase 1 so x
            # transfers come first and matmuls start early) ---
            wq_sb = wpool.tile([128, KC, GE], F32R)
            wk_sb = wpool.tile([128, KC, GE], F32R)
            wv_sb = wpool.tile([128, KC, GE], F32R)
            wp_sb = wpool.tile([128, 2, E], F32R)
            tri_sb = wpool.tile([KB, KB], F32R)

            def load_weight_chunk(kc, which):
                for w_sb, w_d in which:
                    nc.sync.dma_start(
                        out=w_sb[:, kc, :],
                        in_=w_d.bitcast(F32R)[kc * 128:(kc + 1) * 128, :])

            qt_sb = qkpool.tile([128, 2, T], F32R)   # pair-stacked Q^T
            kt_sb = qkpool.tile([128, 2, T], F32R)   # pair-stacked K^T
            v_sb = vpool.tile([128, T // KB, GH * (D + 1)], F32R)  # [k-part, kblock, head-slot(64 V + 1 ones)]

            # ones columns of the augmented V (col D of each 65-wide head
            # slot); loaded after the first x chunks to keep startup clean
            v_ones = v_sb.rearrange("p b (h c) -> p (b h) c", c=D + 1)[:, :, D:D + 1]
            ones_sb = wpool.tile([128, (T // KB) * GH], F32R)
            nc.sync.dma_start(out=ones_sb[:], in_=ones_d.bitcast(F32R))
            nc.vector.tensor_copy(
                out=v_ones,
                in_=ones_sb.rearrange("p (n o) -> p n o", o=1),
            )

            # --- phase 1: projections ---
            proj_ctx = ExitStack()
            qk_ps = proj_ctx.enter_context(tc.tile_pool(name="qk_ps", bufs=o.get("qk_bufs", 2), space="PSUM"))
            v_ps = proj_ctx.enter_context(tc.tile_pool(name="v_ps", bufs=o.get("v_bufs", 2), space="PSUM"))
            for tch in range(NTCH):
                ts0 = tch * TCH
                xts = []
                split0 = o.get("x_split_first", False) and tch == 0
                for kc in range(KC):
                    xt = xpool.tile([128, TCH], F32R, tag="xt")
                    if split0:
                        # halved transfers so the first matmuls start sooner
                        for hf in range(2):
                            nc.sync.dma_start(
                                out=xt[:, hf * (TCH // 2):(hf + 1) * (TCH // 2)],
                                in_=xt_d.bitcast(F32R)[kc * 128:(kc + 1) * 128,
                                                       ts0 + hf * (TCH // 2):ts0 + (hf + 1) * (TCH // 2)])
                    else:
                        nc.sync.dma_start(out=xt[:], in_=xt_d.bitcast(F32R)[kc * 128:(kc + 1) * 128, ts0:ts0 + TCH])
                    xts.append(xt)
                    if tch == 0:
                        # q/k weights ride along with their x chunk; v weights
                        # (used later in the t-chunk) trail by 4 chunks
                        load_weight_chunk(kc, ((wq_sb, wqt_d), (wk_sb, wkt_d)))
                        if kc >= 4:
                            load_weight_chunk(kc - 4, ((wv_sb, wvt_d),))
                if tch == 0:
                    for kc in range(4, KC):
                        load_weight_chunk(kc, ((wv_sb, wvt_d),))
                    nc.sync.dma_start(out=tri_sb[:], in_=tri_d.bitcast(F32R))
                    nc.sync.dma_start(out=wp_sb[:], in_=wpt_d.bitcast(F32R).rearrange("(c p) n -> p c n", p=128))
                halves = ((0, TCH // 2), (TCH // 2, TCH)) if split0 else ((0, TCH),)
                for pair in range(2):
                    psl = slice(pair * 128, (pair + 1) * 128)
                    qp = qk_ps.tile([128, TCH], F32)
                    for h0, h1 in halves:
                        for kc in range(KC):
                            nc.tensor.matmul(qp[:, h0:h1], wq_sb[:, kc, psl], xts[kc][:, h0:h1],
                                             start=(kc == 0), stop=(kc == KC - 1))
                    nc.vector.tensor_copy(out=qt_sb[:, pair, ts0:ts0 + TCH], in_=qp[:])
                    kp = qk_ps.tile([128, TCH], F32)
                    for h0, h1 in halves:
                        for kc in range(KC):
                            nc.tensor.matmul(kp[:, h0:h1], wk_sb[:, kc, psl], xts[kc][:, h0:h1],
                                             start=(kc == 0), stop=(kc == KC - 1))
                    nc.vector.tensor_copy(out=kt_sb[:, pair, ts0:ts0 + TCH], in_=kp[:])
                for tsub in range(TCH // KB):
                    vp = v_ps.tile([128, GE], F32)
                    for kc in range(KC):
                        nc.tensor.matmul(vp[:], xts[kc][:, tsub * KB:(tsub + 1) * KB], wv_sb[:, kc, :],
                                         start=(kc == 0), stop=(kc == KC - 1))
                    tb = tch * (TCH // KB) + tsub
                    nc.vector.tensor_copy(
                        out=v_sb[:, tb, :].rearrange("p (h c) -> p h c", c=D + 1)[:, :, 0:D],
                        in_=vp.rearrange("p (h c) -> p h c", c=D),
                    )

            proj_ctx.close()

            # --- phase 2+3: attention per q-block, then its output projection ---
            attn_ctx = ExitStack()
            s_ps = attn_ctx.enter_context(tc.tile_pool(name="s_ps", bufs=o["s_bufs"], space="PSUM"))
            o_ps = attn_ctx.enter_context(tc.tile_pool(name="o_ps", bufs=o["o_bufs"], space="PSUM"))
            if o["y_in_s"]:
                y_ps, y_tag = s_ps, "s"
            elif o.get("y_in_o", False):
                y_ps, y_tag = o_ps, "o"
            else:
                y_ps = attn_ctx.enter_context(
                    tc.tile_pool(name="y_ps", bufs=o.get("y_bufs", 1), space="PSUM"))
                y_tag = "y"

            def slot(hb):
                return slice(hb * (D + 1), (hb + 1) * (D + 1))

            def tri_eng(pt, r):
                eng = nc.gpsimd if o.get("tri_gpsimd", False) else nc.vector
                eng.tensor_mul(pt[:, r:r + KB], pt[:, r:r + KB], tri_sb[:])

            def normalize(o_p, onorm, h, splits=1):
                # reciprocal of the l row (PSUM partition 64 -> SBUF partition
                # 0; DVE handles the base shift), gpsimd-broadcast across 64
                # partitions, then normalize straight out of PSUM into the
                # pair-stacked onorm tile (head B writes partitions 64:128).
                # splits>1 chops the chain along q so downstream Y matmuls
                # start on the first q-tile sooner (used for the last q-block).
                w = QB // splits
                for s in range(splits):
                    qs = slice(s * w, (s + 1) * w)
                    strip = lpool.tile([1, w], F32, tag="strip")
                    if o.get("recip_fast", True):
                        nc.vector.reciprocal_approx_fast(out=strip[:], in_=o_p[D:D + 1, qs])
                    else:
                        nc.vector.reciprocal(out=strip[:], in_=o_p[D:D + 1, qs])
                    lb = lpool.tile([D, w], F32, tag="lb")
                    nc.gpsimd.partition_broadcast(lb[:], strip[:])
                    nc.vector.tensor_mul(onorm[h * D:(h + 1) * D, qs], o_p[0:D, qs], lb[:])

            for qb in o.get("qb_order", list(range(NQB))):
                q0 = qb * QB
                nk = (q0 + QB) // KB          # kblocks 0..nk-1; last 4 are diagonal
                nfull = nk - 4
                onorms = []
                for pair in range(2):
                    onorm = onpool.tile([128, QB], F32R)
                    heads = [0, 1] if o["interleave"] else None
                    if o["interleave"]:
                        o_ps_t = [o_ps.tile([D + 1, QB], F32, tag="o", name="o_t") for _ in range(2)]
                        qr = [qt_sb[h * D:(h + 1) * D, pair, q0:q0 + QB] for h in range(2)]
                        # full k-blocks two at a time; S matmuls for the two
                        # heads adjacent (disjoint PE row groups -> HW overlap)
                        for j2 in range(0, nfull, 2):
                            sps = [s_ps.tile([128, 2 * QB], F32, tag="s", name="s_t") for _ in range(2)]
                            for jj in range(2):
                                j = j2 + jj
                                for h in range(2):
                                    nc.tensor.matmul(
                                        sps[h][:, jj * QB:(jj + 1) * QB],
                                        kt_sb[h * D:(h + 1) * D, pair, j * KB:(j + 1) * KB],
                                        qr[h], start=True, stop=True)
                            pts = []
                            for h in range(2):
                                pt = ppool.tile([128, 2 * QB], F32R, tag="p")
                                nc.scalar.activation(out=pt[:], in_=sps[h][:], func=EXP, scale=SCALE)
                                pts.append(pt)
                            for jj in range(2):
                                j = j2 + jj
                                for h in range(2):
                                    nc.tensor.matmul(
                                        o_ps_t[h][:], v_sb[:, j, slot(pair * 2 + h)],
                                        pts[h][:, jj * QB:(jj + 1) * QB],
                                        start=(j == 0), stop=False)
                        for j in range(nfull, nk):
                            r = (j - nfull) * KB
                            sps = [s_ps.tile([128, 2 * QB], F32, tag="s", name="s_t") for _ in range(2)]
                            for h in range(2):
                                nc.tensor.matmul(
                                    sps[h][:, 0:QB],
                                    kt_sb[h * D:(h + 1) * D, pair, j * KB:(j + 1) * KB],
                                    qr[h], start=True, stop=True)
                            for h in range(2):
                                pt = ppool.tile([128, 2 * QB], F32R, tag="p")
                                nc.scalar.activation(out=pt[:, r:QB], in_=sps[h][:, r:QB], func=EXP, scale=SCALE)
                                tri_eng(pt, r)
                                nc.tensor.matmul(
                                    o_ps_t[h][:, r:QB], v_sb[:, j, slot(pair * 2 + h)],
                                    pt[:, r:QB], start=(j == 0), stop=(j == nk - 1))
                        for h in range(2):
                            normalize(o_ps_t[h], onorm, h)
                    else:
                        for h in range(2):
                            hb = pair * 2 + h
                            bsl = slice(h * D, h * D + D)
                            o_p = o_ps.tile([D + 1, QB], F32)
                            qrhs = qt_sb[bsl, pair, q0:q0 + QB]
                            for j2 in range(0, nfull, 2):
                                sp = s_ps.tile([128, 2 * QB], F32, tag="s")
                                for jj in range(2):
                                    j = j2 + jj
                                    nc.tensor.matmul(sp[:, jj * QB:(jj + 1) * QB],
                                                     kt_sb[bsl, pair, j * KB:(j + 1) * KB],
                                                     qrhs, start=True, stop=True)
                                pt = ppool.tile([128, 2 * QB], F32R, tag="p")
                                nc.scalar.activation(out=pt[:], in_=sp[:], func=EXP, scale=SCALE)
                                for jj in range(2):
                                    j = j2 + jj
                                    nc.tensor.matmul(o_p[:], v_sb[:, j, slot(hb)],
                                                     pt[:, jj * QB:(jj + 1) * QB],
                                                     start=(j == 0), stop=False)
                            for j2 in range(nfull, nk, 2):
                                # two diagonal k-blocks share one 2-bank psum
                                # tile and a single exp over [r0 : QB+r1+KB]
                                # (the [QB : QB+r1) strip is unread garbage)
                                r0 = (j2 - nfull) * KB
                                r1 = r0 + KB
                                sp = s_ps.tile([128, 2 * QB], F32, tag="s")
                                for jj in range(2):
                                    j = j2 + jj
                                    nc.tensor.matmul(sp[:, jj * QB:(jj + 1) * QB],
                                                     kt_sb[bsl, pair, j * KB:(j + 1) * KB],
                                                     qrhs, start=True, stop=True)
                                pt = ppool.tile([128, 2 * QB], F32R, tag="p")
                                nc.scalar.activation(out=pt[:, r0:2 * QB], in_=sp[:, r0:2 * QB],
                                                     func=EXP, scale=SCALE)
                                for jj, r in ((0, r0), (1, r1)):
                                    j = j2 + jj
                                    base_c = jj * QB
                                    nc.vector.tensor_mul(pt[:, base_c + r:base_c + r + KB],
                                                         pt[:, base_c + r:base_c + r + KB], tri_sb[:])
                                    nc.tensor.matmul(o_p[:, r:QB], v_sb[:, j, slot(hb)],
                                                     pt[:, base_c + r:base_c + QB],
                                                     start=(j == 0), stop=(j == nk - 1))
                            normalize(o_p, onorm, h,
                                      splits=(o.get("norm_splits_last", 1) if qb == NQB - 1 else 1))
                    onorms.append(onorm)
                # output projection for this q-block
                for qt in range(QB // 128):
                    if o.get("y_split", False):
                        yt = ypool.tile([128, E], F32)
                        for nh in range(2):
                            if o.get("y_in_o", False):
                                yp = o_ps.tile([128, 512], F32, tag="o" if o["interleave"] else "o_p", name="yp")
                            elif o.get("y_last_in_s", False) and qb == NQB - 1:
                                yp = s_ps.tile([128, 512], F32, tag="s", name="yp")
                            else:
                                yp = y_ps.tile([128, 512], F32, tag=y_tag, name="yp")
                            for pair in range(2):
                                nc.tensor.matmul(yp[:],
                                                 onorms[pair][:, qt * 128:(qt + 1) * 128],
                                                 wp_sb[:, pair, nh * 512:(nh + 1) * 512],
                                                 start=(pair == 0), stop=(pair == 1))
                            if o.get("y_copy_act", False) and nh == 1:
                                nc.scalar.activation(out=yt[:, nh * 512:(nh + 1) * 512], in_=yp[:],
                                                     func=mybir.ActivationFunctionType.Copy)
                            else:
                                nc.vector.tensor_copy(out=yt[:, nh * 512:(nh + 1) * 512], in_=yp[:])
                        nc.sync.dma_start(out=y_d[q0 + qt * 128:q0 + (qt + 1) * 128, :], in_=yt[:])
                    else:
                        yp = y_ps.tile([128, E], F32, tag=y_tag)
                        for nh in range(2):
                            for pair in range(2):
                                nc.tensor.matmul(yp[:, nh * 512:(nh + 1) * 512],
                                                 onorms[pair][:, qt * 128:(qt + 1) * 128],
                                                 wp_sb[:, pair, nh * 512:(nh + 1) * 512],
                                                 start=(pair == 0), stop=(pair == 1))
                        yt = ypool.tile([128, E], F32)
                        nc.vector.tensor_copy(out=yt[:], in_=yp[:])
                        nc.sync.dma_start(out=y_d[q0 + qt * 128:q0 + (qt + 1) * 128, :], in_=yt[:])
            attn_ctx.close()

    nc.compile()
    return nc


_NC = {}


def _get_program(opts=None):
    key = tuple(sorted((opts or {}).items()))
    if key not in _NC:
        _NC[key] = build_program(opts)
    return _NC[key]


def _make_in_maps(x, Wq, Wk, Wv, Wp):
    x = np.asarray(x, dtype=np.float32)
    wqt = np.ascontiguousarray(np.asarray(Wq, np.float32).T)
    wkt = np.ascontiguousarray(np.asarray(Wk, np.float32).T)
    wvt = np.ascontiguousarray(np.asarray(Wv, np.float32).T)
    wpt = np.ascontiguousarray(np.asarray(Wp, np.float32).T)
    tri = (np.arange(KB)[:, None] <= np.arange(KB)[None, :]).astype(np.float32)
    ones = np.ones((128, (T // KB) * GH), np.float32)
    in_maps = []
    for c in range(N_CORES):
        b, hg = c // 4, c % 4
        in_maps.append({
            "xt": np.ascontiguousarray(x[b].T),
            "wqt": np.ascontiguousarray(wqt[:, hg * GE:(hg + 1) * GE]),
            "wkt": np.ascontiguousarray(wkt[:, hg * GE:(hg + 1) * GE]),
            "wvt": np.ascontiguousarray(wvt[:, hg * GE:(hg + 1) * GE]),
            "wpt": np.ascontiguousarray(wpt[hg * GE:(hg + 1) * GE, :]),
            "tri": tri,
            "ones": ones,
        })
    return in_maps


def run_cores(x, Wq, Wk, Wv, Wp, bp, **spmd_kwargs):
    """Run the 8-core program; returns (y_full, BassKernelResults)."""
    nc = _get_program()
    in_maps = _make_in_maps(x, Wq, Wk, Wv, Wp)
    res = run_bass_kernel_spmd(nc, in_maps, list(range(N_CORES)), **spmd_kwargs)
    parts = [res.results[c]["y"] for c in range(N_CORES)]
    y = np.empty((B, T, E), np.float32)
    for b in range(B):
        acc = parts[4 * b].astype(np.float32)
        for hg in range(1, 4):
            acc = acc + parts[4 * b + hg]
        y[b] = acc
    y += np.asarray(bp, np.float32)[None, None, :]
    return y, res


def kernel(x, Wq, Wk, Wv, Wp, bp):
    y, _ = run_cores(x, Wq, Wk, Wv, Wp, bp)
    return y


def bench(x, Wq, Wk, Wv, Wp, bp, iters=12):
    """Time repeated on-device executions of the compiled program.

    Returns (y_full, list_of_call_seconds). Builds the sharded jit once;
    inputs are device-resident; fresh donated zero outputs are staged
    outside the timed region each iteration.
    """
    import time

    import jax
    import numpy as np_
    from jax.experimental.shard_map import shard_map
    from jax.sharding import Mesh, NamedSharding, PartitionSpec

    from concourse import bass2jax, mybir as mb

    nc = _get_program()
    in_maps = _make_in_maps(x, Wq, Wk, Wv, Wp)
    n_cores = N_CORES
    bass2jax.install_neuronx_cc_hook()

    partition_name = nc.partition_id_tensor.name if nc.partition_id_tensor else None
    in_names, out_names, out_avals, zero_outs = [], [], [], []
    for alloc in nc.m.functions[0].allocations:
        if not isinstance(alloc, mb.MemoryLocationSet):
            continue
        name = alloc.memorylocations[0].name
        if alloc.kind == "ExternalInput":
            if name != partition_name:
                in_names.append(name)
        elif alloc.kind == "ExternalOutput":
            out_names.append(name)
            shape = tuple(alloc.tensor_shape)
            dtype = mb.dt.np(alloc.dtype)
            out_avals.append(jax.core.ShapedArray(shape, dtype))
            zero_outs.append(np_.zeros(shape, dtype))
    n_params = len(in_names)
    all_in_names = in_names + out_names
    if partition_name is not None:
        all_in_names = all_in_names + [partition_name]

    def _body(*args):
        operands = list(args)
        if partition_name is not None:
            operands.append(bass2jax.partition_id_tensor())
        outs = bass2jax._bass_exec_p.bind(
            *operands,
            out_avals=tuple(out_avals),
            in_names=tuple(all_in_names),
            out_names=tuple(out_names),
            lowering_input_output_aliases=(),
            sim_require_finite=True,
            sim_require_nnan=True,
            nc=nc,
        )
        return tuple(outs)

    devices = jax.devices()[:n_cores]
    mesh = Mesh(np_.asarray(devices), ("core",))
    donate = tuple(range(n_params, n_params + len(out_names)))
    sharded = jax.jit(
        shard_map(_body, mesh=mesh,
                  in_specs=(PartitionSpec("core"),) * (n_params + len(out_names)),
                  out_specs=(PartitionSpec("core"),) * len(out_names),
                  check_rep=False),
        donate_argnums=donate, keep_unused=True,
    )
    sh = NamedSharding(mesh, PartitionSpec("core"))
    concat_in = [
        jax.device_put(
            np_.concatenate([np_.asarray(in_maps[c][nm]) for c in range(n_cores)], axis=0), sh)
        for nm in in_names
    ]
    zeros_np = [np_.zeros((n_cores * z.shape[0], *z.shape[1:]), z.dtype) for z in zero_outs]

    times = []
    out_arrs = None
    for it in range(iters):
        dz = [jax.device_put(z, sh) for z in zeros_np]
        jax.block_until_ready(dz)
        t0 = time.perf_counter()
        out_arrs = sharded(*concat_in, *dz)
        jax.block_until_ready(out_arrs)
        times.append(time.perf_counter() - t0)

    parts = [
        np_.asarray(out_arrs[i]).reshape(n_cores, *out_avals[i].shape)
        for i, nm in enumerate(out_names)
    ]
    yi = out_names.index("y")
    y = np_.empty((B, T, E), np_.float32)
    for b in range(B):
        acc = parts[yi][4 * b].astype(np_.float32)
        for hg in range(1, 4):
            acc = acc + parts[yi][4 * b + hg]
        y[b] = acc
    y += np_.asarray(bp, np_.float32)[None, None, :]
    return y, times



# revision 30
# speedup vs baseline: 1.2086x; 1.2086x over previous
"""Multi-head causal attention (B=2, T=2048, E=1024, H=16, D=64) on 8 trn2 cores.

Sharding: core c -> batch b = c // 4, head-group hg = c % 4 (4 heads each).
Per-core: QKV projections for its 4 heads, causal flash attention in
transposed-score layout (S^T[k,q]; softmax denominator folded into a
ones-augmented V matmul), row-parallel output projection producing a partial
[T, E] output. Host sums the 4 partials per batch and adds the bias.

Emission is software-pipelined: the attention stream for q-block qb
(S -> exp -> O, with S/exp skewed one unit ahead of O so the PE never
waits on the Activation engine) is interleaved with the projection
matmuls for t-chunk qb+1 and the output-projection matmuls of earlier
q-blocks, keeping the PE queue saturated while exp runs on Act.
"""
import sys
from collections import deque
from contextlib import ExitStack

sys.path.insert(0, "/opt/trn_rl_repo")

import numpy as np

import concourse.bass as bass
import concourse.tile as tile
from concourse import bacc, mybir
from concourse.bass_utils import run_bass_kernel_spmd

F32 = mybir.dt.float32
F32R = mybir.dt.float32r
BF16 = mybir.dt.bfloat16
EXP = mybir.ActivationFunctionType.Exp

B, T, E, H = 2, 2048, 1024, 16
D = E // H              # 64
N_CORES = 8
GH = 4                  # heads per core
GE = GH * D             # 256 per-core projection width
SCALE = float(D) ** -0.5

TCH = 512               # projection t-chunk
NTCH = T // TCH         # 4
KC = 8                  # contraction chunks of 128 over E
QB = 512                # attention q-block
NQB = T // QB           # 4
KB = 128                # attention k-block


DEFAULT_OPTS = dict(
    s_bufs=2,            # S psum slots ([128,1024] = 2 banks each)
    o_bufs=2,
    mix_bufs=2,          # shared q/k/v/y psum slots ([128,512] = 1 bank each)
    p_bufs=6,
    x_bufs=2,            # [128, KC, TCH] x tiles (one per t-chunk)
    on_bufs=8,
    yst_bufs=4,
    l_bufs=6,
    norm_splits_last=4,
    recip_fast=False,
)


def build_program(opts=None):
    o = dict(DEFAULT_OPTS)
    if opts:
        o.update(opts)
    nc = bacc.Bacc("TRN2", target_bir_lowering=False, debug=False, num_devices=N_CORES)

    xt_d = nc.dram_tensor("xt", [E, T], BF16, kind="ExternalInput").ap()
    wqt_d = nc.dram_tensor("wqt", [E, GE], BF16, kind="ExternalInput").ap()
    wkt_d = nc.dram_tensor("wkt", [E, GE], BF16, kind="ExternalInput").ap()
    wvt_d = nc.dram_tensor("wvt", [E, GE], BF16, kind="ExternalInput").ap()
    wpt_d = nc.dram_tensor("wpt", [GE, E], F32, kind="ExternalInput").ap()
    tri_d = nc.dram_tensor("tri", [KB, 3 * KB], F32, kind="ExternalInput").ap()
    ones_d = nc.dram_tensor("ones", [128, (T // KB) * GH], F32, kind="ExternalInput").ap()
    y_d = nc.dram_tensor("y", [T, E], BF16, kind="ExternalOutput").ap()

    with tile.TileContext(nc) as tc:
        with tc.tile_pool(name="weights", bufs=1) as wpool, \
             tc.tile_pool(name="qk", bufs=1) as qkpool, \
             tc.tile_pool(name="vsb", bufs=1) as vpool, \
             tc.tile_pool(name="xin", bufs=o["x_bufs"]) as xpool, \
             tc.tile_pool(name="ptile", bufs=o["p_bufs"]) as ppool, \
             tc.tile_pool(name="lbc", bufs=o["l_bufs"]) as lpool, \
             tc.tile_pool(name="onorm", bufs=o["on_bufs"]) as onpool, \
             tc.tile_pool(name="ystage", bufs=o["yst_bufs"]) as ypool, \
             tc.tile_pool(name="mix_ps", bufs=o["mix_bufs"], space="PSUM") as mix_ps, \
             tc.tile_pool(name="s_ps", bufs=o["s_bufs"], space="PSUM") as s_ps, \
             tc.tile_pool(name="o_ps", bufs=o["o_bufs"], space="PSUM") as o_ps:

            wq_sb = wpool.tile([128, KC, GE], BF16)
            wk_sb = wpool.tile([128, KC, GE], BF16)
            wv_sb = wpool.tile([128, KC, GE], BF16)
            wp_sb = wpool.tile([128, 2, E], F32R)
            tri_sb = wpool.tile([KB, 3 * KB], F32R)

            qt_sb = qkpool.tile([128, 2, T], F32R)   # pair-stacked Q^T
            kt_sb = qkpool.tile([128, 2, T], F32R)   # pair-stacked K^T
            v_sb = vpool.tile([128, T // KB, GH * (D + 1)], F32R)

            def w_dma(w_sb, w_d, c0, c1):
                nc.sync.dma_start(
                    out=w_sb[:, c0:c1, :],
                    in_=w_d.rearrange("(c p) n -> p c n", p=128)[:, c0:c1, :])

            def start_proj(tch):
                """Emit x DMA(s) for t-chunk tch; return the emission generator."""
                xt = xpool.tile([128, KC, TCH], BF16, tag="xt", name="xt")
                ts0 = tch * TCH
                if tch == 0:
                    # startup: small transfers interleaved in need-order so
                    # the first matmuls begin as early as possible
                    def x_chunk(kc):
                        nc.sync.dma_start(out=xt[:, kc, :],
                                          in_=xt_d[kc * 128:(kc + 1) * 128, ts0:ts0 + TCH])
                    w_dma(wq_sb, wqt_d, 0, 2)
                    x_chunk(0)
                    x_chunk(1)
                    w_dma(wq_sb, wqt_d, 2, 4)
                    x_chunk(2)
                    x_chunk(3)
                    w_dma(wq_sb, wqt_d, 4, 6)
                    x_chunk(4)
                    w_dma(wq_sb, wqt_d, 6, 8)
                    x_chunk(5)
                    w_dma(wk_sb, wkt_d, 0, 4)
                    x_chunk(6)
                    w_dma(wk_sb, wkt_d, 4, 8)
                    x_chunk(7)
                    w_dma(wv_sb, wvt_d, 0, KC)
                    nc.sync.dma_start(out=tri_sb[:], in_=tri_d.bitcast(F32R))
                    nc.sync.dma_start(out=wp_sb[:], in_=wpt_d.bitcast(F32R).rearrange("(c p) n -> p c n", p=128))
                else:
                    nc.sync.dma_start(
                        out=xt[:],
                        in_=xt_d.rearrange("(c p) t -> p c t", p=128)[:, :, ts0:ts0 + TCH])
                return proj_gen(tch, xt)

            def proj_gen(tch, xt):
                ts0 = tch * TCH
                for pair in range(2):
                    psl = slice(pair * 128, (pair + 1) * 128)
                    qp = mix_ps.tile([128, TCH], F32, tag="mix", name="qp")
                    for kc in range(KC):
                        nc.tensor.matmul(qp[:], wq_sb[:, kc, psl], xt[:, kc, :],
                                         start=(kc == 0), stop=(kc == KC - 1))
                        if kc % 2 == 1 and kc < KC - 1:
                            yield
                    nc.vector.tensor_copy(out=qt_sb[:, pair, ts0:ts0 + TCH], in_=qp[:])
                    yield
                    kp = mix_ps.tile([128, TCH], F32, tag="mix", name="kp")
                    for kc in range(KC):
                        nc.tensor.matmul(kp[:], wk_sb[:, kc, psl], xt[:, kc, :],
                                         start=(kc == 0), stop=(kc == KC - 1))
                        if kc % 2 == 1 and kc < KC - 1:
                            yield
                    nc.vector.tensor_copy(out=kt_sb[:, pair, ts0:ts0 + TCH], in_=kp[:])
                    yield
                for tsub in range(TCH // KB):
                    vp = mix_ps.tile([128, TCH], F32, tag="mix", name="vp")
                    for kc in range(KC):
                        nc.tensor.matmul(vp[:, 0:GE], xt[:, kc, tsub * KB:(tsub + 1) * KB],
                                         wv_sb[:, kc, :],
                                         start=(kc == 0), stop=(kc == KC - 1))
                        if kc == 3:
                            yield
                    tb = tch * (TCH // KB) + tsub
                    nc.vector.tensor_copy(
                        out=v_sb[:, tb, :].rearrange("p (h c) -> p h c", c=D + 1)[:, :, 0:D],
                        in_=vp[:, 0:GE].rearrange("p (h c) -> p h c", c=D),
                    )
                    yield

            def slot(hb):
                return slice(hb * (D + 1), (hb + 1) * (D + 1))

            def normalize(o_p, onorm, h, splits=1):
                w = QB // splits
                for s in range(splits):
                    qs = slice(s * w, (s + 1) * w)
                    strip = lpool.tile([1, w], F32, tag="strip")
                    if o["recip_fast"]:
                        nc.vector.reciprocal_approx_fast(out=strip[:], in_=o_p[D:D + 1, qs])
                    else:
                        nc.vector.reciprocal(out=strip[:], in_=o_p[D:D + 1, qs])
                    lb = lpool.tile([D, w], F32, tag="lb")
                    nc.gpsimd.partition_broadcast(lb[:], strip[:])
                    nc.vector.tensor_mul(onorm[h * D:(h + 1) * D, qs], o_p[0:D, qs], lb[:])

            def emit_S_exp_full(sp, pt, bsl, pair, q0, j2):
                for jj in range(2):
                    j = j2 + jj
                    nc.tensor.matmul(sp[:, jj * QB:(jj + 1) * QB],
                                     kt_sb[bsl, pair, j * KB:(j + 1) * KB],
                                     qt_sb[bsl, pair, q0:q0 + QB],
                                     start=True, stop=True)
                nc.scalar.activation(out=pt[:], in_=sp[:], func=EXP, scale=SCALE)

            def emit_S_exp_diag(sp, pt, bsl, pair, q0, j2, nfull):
                for jj in range(2):
                    j = j2 + jj
                    r = (j - nfull) * KB
                    w = min(r, 256)
                    c0 = jj * QB
                    nc.tensor.matmul(sp[:, c0 + w:c0 + QB],
                                     kt_sb[bsl, pair, j * KB:(j + 1) * KB],
                                     qt_sb[bsl, pair, q0 + w:q0 + QB],
                                     start=True, stop=True)
                    nc.scalar.activation(out=pt[:, c0 + w:c0 + QB],
                                         in_=sp[:, c0 + w:c0 + QB],
                                         func=EXP, scale=SCALE)
                    if r == 384:
                        nc.vector.tensor_mul(pt[:, c0 + 256:c0 + QB],
                                             pt[:, c0 + 256:c0 + QB],
                                             tri_sb[:, KB:3 * KB])
                    else:
                        nc.vector.tensor_mul(pt[:, c0 + r:c0 + r + KB],
                                             pt[:, c0 + r:c0 + r + KB],
                                             tri_sb[:, 0:KB])

            # ---- pipelined emission ----
            gen = start_proj(0)
            for _ in gen:       # projections for t-chunk 0 up front
                pass
            # ones column of the augmented V (softmax denominator)
            v_ones = v_sb.rearrange("p b (h c) -> p (b h) c", c=D + 1)[:, :, D:D + 1]
            ones_sb = wpool.tile([128, (T // KB) * GH], F32R)
            nc.sync.dma_start(out=ones_sb[:], in_=ones_d.bitcast(F32R))
            nc.vector.tensor_copy(out=v_ones,
                                  in_=ones_sb.rearrange("p (n o) -> p n o", o=1))
            gen = start_proj(1)
            gen_rem = 24        # yields per proj generator

            fillers = deque()
            pend = []

            for qb in range(NQB):
                q0 = qb * QB
                nk = (q0 + QB) // KB
                nfull = nk - 4
                onorms = [onpool.tile([128, QB], F32R, tag="onorm", name="onorm")
                          for i in range(2)]
                n_units = 4 * (nfull // 2 + 2)
                slot_i = 0
                for pair in range(2):
                    for h in range(2):
                        hb = pair * 2 + h
                        bsl = slice(h * D, h * D + D)
                        o_p = o_ps.tile([D + 1, QB], F32, tag="o", name="o_p")
                        units = [("full", j2) for j2 in range(0, nfull, 2)] + \
                                [("diag", j2) for j2 in range(nfull, nk, 2)]
                        for ui, (kind, j2) in enumerate(units):
                            last = ui == len(units) - 1
                            sp = s_ps.tile([128, 2 * QB], F32, tag="s", name="sp")
                            pt = ppool.tile([128, 2 * QB], F32R, tag="p", name="pt")
                            if kind == "full":
                                emit_S_exp_full(sp, pt, bsl, pair, q0, j2)

                                def Bc(pt=pt, o_p=o_p, hb=hb, j2=j2):
                                    for jj in range(2):
                                        j = j2 + jj
                                        nc.tensor.matmul(o_p[:], v_sb[:, j, slot(hb)],
                                                         pt[:, jj * QB:(jj + 1) * QB],
                                                         start=(j == 0), stop=False)
                            else:
                                emit_S_exp_diag(sp, pt, bsl, pair, q0, j2, nfull)

                                def Bc(pt=pt, o_p=o_p, hb=hb, j2=j2, nfull=nfull, nk=nk):
                                    for jj in range(2):
                                        j = j2 + jj
                                        r = (j - nfull) * KB
                                        w = min(r, 256)
                                        nc.tensor.matmul(o_p[:, w:QB], v_sb[:, j, slot(hb)],
                                                         pt[:, jj * QB + w:(jj + 1) * QB],
                                                         start=(j == 0), stop=(j == nk - 1))
                            if last:
                                def B2(Bc=Bc, o_p=o_p, onorm=onorms[pair], h=h, qb=qb):
                                    Bc()
                                    normalize(o_p, onorm, h,
                                              splits=(o["norm_splits_last"]
                                                      if qb == NQB - 1 else 1))
                                pend.append(B2)
                            else:
                                pend.append(Bc)
                            # pump proj filler, then flush pending O work
                            if gen is not None:
                                k = max(1, -(-gen_rem // (n_units - slot_i)))
                                for _ in range(k):
                                    try:
                                        next(gen)
                                        gen_rem -= 1
                                    except StopIteration:
                                        gen = None
                                        break
                            while pend:
                                pend.pop(0)()
                            # spread deferred out-projection work over the
                            # remaining proj-less slots
                            if gen is None and fillers and \
                                    len(fillers) * 2 >= n_units - slot_i:
                                fillers.popleft()()
                            slot_i += 1
                # drain any remaining proj work for the next t-chunk
                if gen is not None:
                    for _ in gen:
                        pass
                    gen = None
                if qb + 2 <= NTCH - 1:
                    gen = start_proj(qb + 2)
                    gen_rem = 24

                # out-projection for this q-block, deferred as filler work
                for qt in range(QB // 128):
                    def F(qt=qt, q0=q0, onorms=onorms, use_s=(qb == NQB - 1 and qt % 2 == 1)):
                        yt = ypool.tile([128, E], BF16, tag="yt", name="yt")
                        for nh in range(2):
                            if use_s:
                                yp = s_ps.tile([128, 512], F32, tag="s", name="yp")
                            else:
                                yp = mix_ps.tile([128, 512], F32, tag="mix", name="yp")
                            for pair in range(2):
                                nc.tensor.matmul(yp[:],
                                                 onorms[pair][:, qt * 128:(qt + 1) * 128],
                                                 wp_sb[:, pair, nh * 512:(nh + 1) * 512],
                                                 start=(pair == 0), stop=(pair == 1))
                            nc.vector.tensor_copy(out=yt[:, nh * 512:(nh + 1) * 512], in_=yp[:])
                            nc.sync.dma_start(
                                out=y_d[q0 + qt * 128:q0 + (qt + 1) * 128, nh * 512:(nh + 1) * 512],
                                in_=yt[:, nh * 512:(nh + 1) * 512])
                    fillers.append(F)
            while pend:
                pend.pop(0)()
            while fillers:
                fillers.popleft()()

    nc.compile()
    return nc


_NC = {}


def _get_program(opts=None):
    key = tuple(sorted((opts or {}).items()))
    if key not in _NC:
        _NC[key] = build_program(opts)
    return _NC[key]


def _make_in_maps(x, Wq, Wk, Wv, Wp):
    bf16 = mybir.dt.np(BF16)
    x = np.asarray(x, dtype=np.float32).astype(bf16)
    wqt = np.ascontiguousarray(np.asarray(Wq, np.float32).T.astype(bf16))
    wkt = np.ascontiguousarray(np.asarray(Wk, np.float32).T.astype(bf16))
    wvt = np.ascontiguousarray(np.asarray(Wv, np.float32).T.astype(bf16))
    wpt = np.ascontiguousarray(np.asarray(Wp, np.float32).T)
    tri1 = (np.arange(KB)[:, None] <= np.arange(KB)[None, :]).astype(np.float32)
    tri = np.concatenate([tri1, np.zeros((KB, KB), np.float32), tri1], axis=1)
    in_maps = []
    for c in range(N_CORES):
        b, hg = c // 4, c % 4
        in_maps.append({
            "xt": np.ascontiguousarray(x[b].T),
            "wqt": np.ascontiguousarray(wqt[:, hg * GE:(hg + 1) * GE]),
            "wkt": np.ascontiguousarray(wkt[:, hg * GE:(hg + 1) * GE]),
            "wvt": np.ascontiguousarray(wvt[:, hg * GE:(hg + 1) * GE]),
            "wpt": np.ascontiguousarray(wpt[hg * GE:(hg + 1) * GE, :]),
            "tri": tri,
            "ones": np.ones((128, (T // KB) * GH), np.float32),
        })
    return in_maps


def run_cores(x, Wq, Wk, Wv, Wp, bp, **spmd_kwargs):
    """Run the 8-core program; returns (y_full, BassKernelResults)."""
    nc = _get_program()
    in_maps = _make_in_maps(x, Wq, Wk, Wv, Wp)
    res = run_bass_kernel_spmd(nc, in_maps, list(range(N_CORES)), **spmd_kwargs)
    parts = [np.asarray(res.results[c]["y"]).astype(np.float32) for c in range(N_CORES)]
    y = np.empty((B, T, E), np.float32)
    for b in range(B):
        acc = parts[4 * b]
        for hg in range(1, 4):
            acc = acc + parts[4 * b + hg]
        y[b] = acc
    y += np.asarray(bp, np.float32)[None, None, :]
    return y, res


def kernel(x, Wq, Wk, Wv, Wp, bp):
    y, _ = run_cores(x, Wq, Wk, Wv, Wp, bp)
    return y


def bench(x, Wq, Wk, Wv, Wp, bp, iters=12):
    """Time repeated on-device executions of the compiled program.

    Returns (y_full, list_of_call_seconds). Builds the sharded jit once;
    inputs are device-resident; fresh donated zero outputs are staged
    outside the timed region each iteration.
    """
    import time

    import jax
    import numpy as np_
    from jax.experimental.shard_map import shard_map
    from jax.sharding import Mesh, NamedSharding, PartitionSpec

    from concourse import bass2jax, mybir as mb

    nc = _get_program()
    in_maps = _make_in_maps(x, Wq, Wk, Wv, Wp)
    n_cores = N_CORES
    bass2jax.install_neuronx_cc_hook()

    partition_name = nc.partition_id_tensor.name if nc.partition_id_tensor else None
    in_names, out_names, out_avals, zero_outs = [], [], [], []
    for alloc in nc.m.functions[0].allocations:
        if not isinstance(alloc, mb.MemoryLocationSet):
            continue
        name = alloc.memorylocations[0].name
        if alloc.kind == "ExternalInput":
            if name != partition_name:
                in_names.append(name)
        elif alloc.kind == "ExternalOutput":
            out_names.append(name)
            shape = tuple(alloc.tensor_shape)
            dtype = mb.dt.np(alloc.dtype)
            out_avals.append(jax.core.ShapedArray(shape, dtype))
            zero_outs.append(np_.zeros(shape, dtype))
    n_params = len(in_names)
    all_in_names = in_names + out_names
    if partition_name is not None:
        all_in_names = all_in_names + [partition_name]

    def _body(*args):
        operands = list(args)
        if partition_name is not None:
            operands.append(bass2jax.partition_id_tensor())
        outs = bass2jax._bass_exec_p.bind(
            *operands,
            out_avals=tuple(out_avals),
            in_names=tuple(all_in_names),
            out_names=tuple(out_names),
            lowering_input_output_aliases=(),
            sim_require_finite=True,
            sim_require_nnan=True,
            nc=nc,
        )
        return tuple(outs)

    devices = jax.devices()[:n_cores]
    mesh = Mesh(np_.asarray(devices), ("core",))
    donate = tuple(range(n_params, n_params + len(out_names)))
    sharded = jax.jit(
        shard_map(_body, mesh=mesh,
                  in_specs=(PartitionSpec("core"),) * (n_params + len(out_names)),
                  out_specs=(PartitionSpec("core"),) * len(out_names),
                  check_rep=False),
        donate_argnums=donate, keep_unused=True,
    )
    sh = NamedSharding(mesh, PartitionSpec("core"))
    concat_in = [
        jax.device_put(
            np_.concatenate([np_.asarray(in_maps[c][nm]) for c in range(n_cores)], axis=0), sh)
        for nm in in_names
    ]
    zeros_np = [np_.zeros((n_cores * z.shape[0], *z.shape[1:]), z.dtype) for z in zero_outs]

    times = []
    out_arrs = None
    for it in range(iters):
        dz = [jax.device_put(z, sh) for z in zeros_np]
        jax.block_until_ready(dz)
        t0 = time.perf_counter()
        out_arrs = sharded(*concat_in, *dz)
        jax.block_until_ready(out_arrs)
        times.append(time.perf_counter() - t0)

    parts = [
        np_.asarray(out_arrs[i]).reshape(n_cores, *out_avals[i].shape)
        for i, nm in enumerate(out_names)
    ]
    yi = out_names.index("y")
    y = np_.empty((B, T, E), np_.float32)
    for b in range(B):
        acc = parts[yi][4 * b].astype(np_.float32)
        for hg in range(1, 4):
            acc = acc + parts[yi][4 * b + hg]
        y[b] = acc
    y += np_.asarray(bp, np_.float32)[None, None, :]
    return y, times


# revision 49
# speedup vs baseline: 1.2684x; 1.0495x over previous
"""Multi-head causal attention (B=2, T=2048, E=1024, H=16, D=64) on 8 trn2 cores.

Sharding: core c -> batch b = c // 4, head-group hg = c % 4 (4 heads each).
Per-core: QKV projections for its 4 heads, causal flash attention in
transposed-score layout (S^T[k,q]; softmax denominator folded into a
ones-augmented V matmul), row-parallel output projection producing a partial
[T, E] output. Host sums the 4 partials per batch and adds the bias.

Emission is software-pipelined: the attention stream for q-block qb
(S -> exp -> O, with S/exp skewed one unit ahead of O so the PE never
waits on the Activation engine) is interleaved with the projection
matmuls for t-chunk qb+1 and the output-projection matmuls of earlier
q-blocks, keeping the PE queue saturated while exp runs on Act.
"""
import sys
from collections import deque
from contextlib import ExitStack

sys.path.insert(0, "/opt/trn_rl_repo")

import numpy as np

import concourse.bass as bass
import concourse.tile as tile
from concourse import bacc, mybir
from concourse.bass_utils import run_bass_kernel_spmd

F32 = mybir.dt.float32
F32R = mybir.dt.float32r
BF16 = mybir.dt.bfloat16
EXP = mybir.ActivationFunctionType.Exp
COPY = mybir.ActivationFunctionType.Copy

B, T, E, H = 2, 2048, 1024, 16
D = E // H              # 64
N_CORES = 8
GH = 4                  # heads per core
GE = GH * D             # 256 per-core projection width
SCALE = float(D) ** -0.5

TCH = 512               # projection t-chunk
NTCH = T // TCH         # 4
KC = 8                  # contraction chunks of 128 over E
QB = 512                # attention q-block
NQB = T // QB           # 4
KB = 128                # attention k-block


DEFAULT_OPTS = dict(
    s_bufs=2,            # S psum slots ([128,1024] = 2 banks each)
    o_bufs=2,
    mix_bufs=2,          # shared q/k/v/y psum slots ([128,512] = 1 bank each)
    p_bufs=10,
    x_bufs=2,            # [128, KC, TCH] x tiles (one per t-chunk)
    on_bufs=8,
    yst_bufs=4,
    l_bufs=6,
    norm_splits_last=2,
    recip_fast=False,
    y_halves=True,       # y DMA per 512-wide half vs full tile
    startup=(("wq", 0, 2), ("x", 0, 2), ("wq", 2, 8), ("x", 2, 4),
             ("wk", 0, 8), ("x", 4, 8), ("wv", 0, 8)),
    y_cp_alt=True,       # spread yp->yt copies across Act (tail) / Pool (mid) / DVE
)


def build_program(opts=None):
    o = dict(DEFAULT_OPTS)
    if opts:
        o.update(opts)
    nc = bacc.Bacc("TRN2", target_bir_lowering=False, debug=False, num_devices=N_CORES)

    xt_d = nc.dram_tensor("xt", [E, T], BF16, kind="ExternalInput").ap()
    wqt_d = nc.dram_tensor("wqt", [E, GE], BF16, kind="ExternalInput").ap()
    wkt_d = nc.dram_tensor("wkt", [E, GE], BF16, kind="ExternalInput").ap()
    wvt_d = nc.dram_tensor("wvt", [E, GE], BF16, kind="ExternalInput").ap()
    wpt_d = nc.dram_tensor("wpt", [GE, E], F32, kind="ExternalInput").ap()
    tri_d = nc.dram_tensor("tri", [KB, 3 * KB], F32, kind="ExternalInput").ap()
    ones_d = nc.dram_tensor("ones", [128, (T // KB) * GH], F32, kind="ExternalInput").ap()
    y_d = nc.dram_tensor("y", [T, E], BF16, kind="ExternalOutput").ap()

    with tile.TileContext(nc) as tc:
        with tc.tile_pool(name="weights", bufs=1) as wpool, \
             tc.tile_pool(name="qk", bufs=1) as qkpool, \
             tc.tile_pool(name="vsb", bufs=1) as vpool, \
             tc.tile_pool(name="xin", bufs=o["x_bufs"]) as xpool, \
             tc.tile_pool(name="ptile", bufs=o["p_bufs"]) as ppool, \
             tc.tile_pool(name="lbc", bufs=o["l_bufs"]) as lpool, \
             tc.tile_pool(name="onorm", bufs=o["on_bufs"]) as onpool, \
             tc.tile_pool(name="ystage", bufs=o["yst_bufs"]) as ypool, \
             tc.tile_pool(name="mix_ps", bufs=o["mix_bufs"], space="PSUM") as mix_ps, \
             tc.tile_pool(name="s_ps", bufs=o["s_bufs"], space="PSUM") as s_ps, \
             tc.tile_pool(name="o_ps", bufs=o["o_bufs"], space="PSUM") as o_ps:

            wq_sb = wpool.tile([128, KC, GE], BF16)
            wk_sb = wpool.tile([128, KC, GE], BF16)
            wv_sb = wpool.tile([128, KC, GE], BF16)
            wp_sb = wpool.tile([128, 2, E], F32R)
            tri_sb = wpool.tile([KB, 3 * KB], F32R)

            qt_sb = qkpool.tile([128, 2, T], F32R)   # pair-stacked Q^T
            kt_sb = qkpool.tile([128, 2, T], F32R)   # pair-stacked K^T
            v_sb = vpool.tile([128, T // KB, GH * (D + 1)], F32R)

            def w_dma(w_sb, w_d, c0, c1):
                nc.sync.dma_start(
                    out=w_sb[:, c0:c1, :],
                    in_=w_d.rearrange("(c p) n -> p c n", p=128)[:, c0:c1, :])

            def start_proj(tch):
                """Emit x DMA(s) for t-chunk tch; return the emission generator."""
                xt = xpool.tile([128, KC, TCH], BF16, tag="xt", name="xt")
                ts0 = tch * TCH
                if tch == 0:
                    # startup: transfers interleaved in need-order so the
                    # first matmuls begin as early as possible
                    def x_chunk(c0, c1):
                        nc.sync.dma_start(out=xt[:, c0:c1, :],
                                          in_=xt_d.rearrange("(c p) t -> p c t", p=128)[:, c0:c1, ts0:ts0 + TCH])
                    for kind, c0, c1 in o["startup"]:
                        if kind == "x":
                            x_chunk(c0, c1)
                        elif kind == "wq":
                            w_dma(wq_sb, wqt_d, c0, c1)
                        elif kind == "wk":
                            w_dma(wk_sb, wkt_d, c0, c1)
                        elif kind == "wv":
                            w_dma(wv_sb, wvt_d, c0, c1)
                    nc.sync.dma_start(out=tri_sb[:], in_=tri_d.bitcast(F32R))
                    nc.sync.dma_start(out=wp_sb[:], in_=wpt_d.bitcast(F32R).rearrange("(c p) n -> p c n", p=128))
                else:
                    nc.sync.dma_start(
                        out=xt[:],
                        in_=xt_d.rearrange("(c p) t -> p c t", p=128)[:, :, ts0:ts0 + TCH])
                return proj_gen(tch, xt)

            def proj_gen(tch, xt):
                ts0 = tch * TCH
                for pair in range(2):
                    psl = slice(pair * 128, (pair + 1) * 128)
                    qp = mix_ps.tile([128, TCH], F32, tag="mix", name="qp")
                    for kc in range(KC):
                        nc.tensor.matmul(qp[:], wq_sb[:, kc, psl], xt[:, kc, :],
                                         start=(kc == 0), stop=(kc == KC - 1))
                        if kc % 2 == 1 and kc < KC - 1:
                            yield
                    nc.vector.tensor_copy(out=qt_sb[:, pair, ts0:ts0 + TCH], in_=qp[:])
                    yield
                    kp = mix_ps.tile([128, TCH], F32, tag="mix", name="kp")
                    for kc in range(KC):
                        nc.tensor.matmul(kp[:], wk_sb[:, kc, psl], xt[:, kc, :],
                                         start=(kc == 0), stop=(kc == KC - 1))
                        if kc % 2 == 1 and kc < KC - 1:
                            yield
                    nc.vector.tensor_copy(out=kt_sb[:, pair, ts0:ts0 + TCH], in_=kp[:])
                    yield
                for tsub in range(TCH // KB):
                    vp = mix_ps.tile([128, TCH], F32, tag="mix", name="vp")
                    for kc in range(KC):
                        nc.tensor.matmul(vp[:, 0:GE], xt[:, kc, tsub * KB:(tsub + 1) * KB],
                                         wv_sb[:, kc, :],
                                         start=(kc == 0), stop=(kc == KC - 1))
                        if kc == 3:
                            yield
                    tb = tch * (TCH // KB) + tsub
                    nc.vector.tensor_copy(
                        out=v_sb[:, tb, :].rearrange("p (h c) -> p h c", c=D + 1)[:, :, 0:D],
                        in_=vp[:, 0:GE].rearrange("p (h c) -> p h c", c=D),
                    )
                    yield

            def slot(hb):
                return slice(hb * (D + 1), (hb + 1) * (D + 1))

            def normalize(o_p, onorm, h, splits=1, only=None):
                w = QB // splits
                for s in range(splits):
                    if only is not None and s not in only:
                        continue
                    qs = slice(s * w, (s + 1) * w)
                    strip = lpool.tile([1, w], F32, tag="strip")
                    if o["recip_fast"]:
                        nc.vector.reciprocal_approx_fast(out=strip[:], in_=o_p[D:D + 1, qs])
                    else:
                        nc.vector.reciprocal(out=strip[:], in_=o_p[D:D + 1, qs])
                    lb = lpool.tile([D, w], F32, tag="lb")
                    nc.gpsimd.partition_broadcast(lb[:], strip[:])
                    nc.vector.tensor_mul(onorm[h * D:(h + 1) * D, qs], o_p[0:D, qs], lb[:])

            def emit_S_exp_full(sp, pt, bsl, pair, q0, j2):
                for jj in range(2):
                    j = j2 + jj
                    nc.tensor.matmul(sp[:, jj * QB:(jj + 1) * QB],
                                     kt_sb[bsl, pair, j * KB:(j + 1) * KB],
                                     qt_sb[bsl, pair, q0:q0 + QB],
                                     start=True, stop=True)
                nc.scalar.activation(out=pt[:], in_=sp[:], func=EXP, scale=SCALE)

            def emit_S_exp_diag(sp, pt, bsl, pair, q0, j2, nfull):
                for jj in range(2):
                    j = j2 + jj
                    r = (j - nfull) * KB
                    w = min(r, 256)
                    c0 = jj * QB
                    nc.tensor.matmul(sp[:, c0 + w:c0 + QB],
                                     kt_sb[bsl, pair, j * KB:(j + 1) * KB],
                                     qt_sb[bsl, pair, q0 + w:q0 + QB],
                                     start=True, stop=True)
                    nc.scalar.activation(out=pt[:, c0 + w:c0 + QB],
                                         in_=sp[:, c0 + w:c0 + QB],
                                         func=EXP, scale=SCALE)
                    if r == 384:
                        nc.vector.tensor_mul(pt[:, c0 + 256:c0 + QB],
                                             pt[:, c0 + 256:c0 + QB],
                                             tri_sb[:, KB:3 * KB])
                    else:
                        nc.vector.tensor_mul(pt[:, c0 + r:c0 + r + KB],
                                             pt[:, c0 + r:c0 + r + KB],
                                             tri_sb[:, 0:KB])

            # ---- pipelined emission ----
            gen = start_proj(0)
            for _ in gen:       # projections for t-chunk 0 up front
                pass
            # ones column of the augmented V (softmax denominator)
            v_ones = v_sb.rearrange("p b (h c) -> p (b h) c", c=D + 1)[:, :, D:D + 1]
            ones_sb = wpool.tile([128, (T // KB) * GH], F32R)
            nc.sync.dma_start(out=ones_sb[:], in_=ones_d.bitcast(F32R))
            nc.vector.tensor_copy(out=v_ones,
                                  in_=ones_sb.rearrange("p (n o) -> p n o", o=1))
            gen = start_proj(1)
            gen_rem = 24        # yields per proj generator

            fillers = deque()
            pend = []

            for qb in range(NQB):
                q0 = qb * QB
                nk = (q0 + QB) // KB
                nfull = nk - 4
                onorms = [onpool.tile([128, QB], F32R, tag="onorm", name="onorm")
                          for i in range(2)]
                n_units = 4 * (nfull // 2 + 2)
                slot_i = 0
                for pair in range(2):
                    for h in range(2):
                        hb = pair * 2 + h
                        bsl = slice(h * D, h * D + D)
                        o_p = o_ps.tile([D + 1, QB], F32, tag="o", name="o_p")
                        units = [("full", j2) for j2 in range(0, nfull, 2)] + \
                                [("diag", j2) for j2 in range(nfull, nk, 2)]
                        for ui, (kind, j2) in enumerate(units):
                            last = ui == len(units) - 1
                            sp = s_ps.tile([128, 2 * QB], F32, tag="s", name="sp")
                            pt = ppool.tile([128, 2 * QB], F32R, tag="p", name="pt")
                            if kind == "full":
                                emit_S_exp_full(sp, pt, bsl, pair, q0, j2)

                                def Bc(pt=pt, o_p=o_p, hb=hb, j2=j2):
                                    for jj in range(2):
                                        j = j2 + jj
                                        nc.tensor.matmul(o_p[:], v_sb[:, j, slot(hb)],
                                                         pt[:, jj * QB:(jj + 1) * QB],
                                                         start=(j == 0), stop=False)
                            else:
                                emit_S_exp_diag(sp, pt, bsl, pair, q0, j2, nfull)

                                def Bc(pt=pt, o_p=o_p, hb=hb, j2=j2, nfull=nfull, nk=nk):
                                    for jj in range(2):
                                        j = j2 + jj
                                        r = (j - nfull) * KB
                                        w = min(r, 256)
                                        nc.tensor.matmul(o_p[:, w:QB], v_sb[:, j, slot(hb)],
                                                         pt[:, jj * QB + w:(jj + 1) * QB],
                                                         start=(j == 0), stop=(j == nk - 1))
                            if last:
                                def B2(Bc=Bc, o_p=o_p, onorm=onorms[pair], h=h, qb=qb):
                                    Bc()
                                    if qb == NQB - 1:
                                        # cols [0:256] were normalized a unit
                                        # early (see below); finish [256:512]
                                        normalize(o_p, onorm, h, splits=2, only=(1,))
                                    else:
                                        normalize(o_p, onorm, h)
                                nxt = B2
                            elif qb == NQB - 1 and kind == "diag":
                                # first diagonal pair: o_p[:, 0:256] is final
                                # (later diag blocks only write [256:512]), so
                                # its normalize can start one unit early
                                def B2a(Bc=Bc, o_p=o_p, onorm=onorms[pair], h=h):
                                    Bc()
                                    normalize(o_p, onorm, h, splits=2, only=(0,))
                                nxt = B2a
                            else:
                                nxt = Bc
                            # pump proj filler, flush the PREVIOUS unit's O
                            # work (one-unit skew hides the exp latency), then
                            # queue this unit's O for the next slot
                            if gen is not None:
                                k = max(1, -(-gen_rem // (n_units - slot_i)))
                                for _ in range(k):
                                    try:
                                        next(gen)
                                        gen_rem -= 1
                                    except StopIteration:
                                        gen = None
                                        break
                            while pend:
                                pend.pop(0)()
                            pend.append(nxt)
                            # spread deferred out-projection work over the
                            # remaining proj-less slots
                            if gen is None and fillers and \
                                    len(fillers) * 2 >= n_units - slot_i:
                                fillers.popleft()()
                            slot_i += 1
                # drain any remaining proj work for the next t-chunk
                if gen is not None:
                    for _ in gen:
                        pass
                    gen = None
                if qb + 2 <= NTCH - 1:
                    gen = start_proj(qb + 2)
                    gen_rem = 24

                # out-projection for this q-block, deferred as filler work
                yt2_box = [None]
                for qt in range(QB // 128):
                    def F(qt=qt, q0=q0, onorms=onorms, alt=(qb == NQB - 1), yt2_box=yt2_box):
                        if alt:
                            # last q-block: pairs of row-tiles share one
                            # staging tile + a single batched DMA, shrinking
                            # the per-DMA dispatch tail
                            if qt % 2 == 0:
                                yt2_box[0] = ypool.tile([128, 2, E], BF16, tag="yt", name="yt")
                            yt = yt2_box[0][:, qt % 2, :]
                        else:
                            yt = ypool.tile([128, E], BF16, tag="yt", name="yt")
                        for nh in range(2):
                            if alt and (qt * 2 + nh) % 2 == 1:
                                yp = s_ps.tile([128, 512], F32, tag="s", name="yp")
                            else:
                                yp = mix_ps.tile([128, 512], F32, tag="mix", name="yp")
                            for pair in range(2):
                                nc.tensor.matmul(yp[:],
                                                 onorms[pair][:, qt * 128:(qt + 1) * 128],
                                                 wp_sb[:, pair, nh * 512:(nh + 1) * 512],
                                                 start=(pair == 0), stop=(pair == 1))
                            yh = yt[:, nh * 512:(nh + 1) * 512]
                            if o["y_cp_alt"] and alt and (qt * 2 + nh) % 2 == 0:
                                nc.scalar.activation(out=yh, in_=yp[:], func=COPY)
                            else:
                                nc.vector.tensor_copy(out=yh, in_=yp[:])
                            if not alt and o["y_halves"]:
                                nc.sync.dma_start(
                                    out=y_d[q0 + qt * 128:q0 + (qt + 1) * 128, nh * 512:(nh + 1) * 512],
                                    in_=yt[:, nh * 512:(nh + 1) * 512])
                        if alt and qt % 2 == 1:
                            nc.sync.dma_start(
                                out=y_d[q0 + (qt - 1) * 128:q0 + (qt + 1) * 128, :].rearrange(
                                    "(c p) n -> p c n", p=128),
                                in_=yt2_box[0][:])
                        elif not alt and not o["y_halves"]:
                            nc.sync.dma_start(out=y_d[q0 + qt * 128:q0 + (qt + 1) * 128, :], in_=yt[:])
                    fillers.append(F)
            while pend:
                pend.pop(0)()
            while fillers:
                fillers.popleft()()

    nc.compile()
    return nc


_NC = {}


def _get_program(opts=None):
    key = tuple(sorted((opts or {}).items()))
    if key not in _NC:
        _NC[key] = build_program(opts)
    return _NC[key]


def _make_in_maps(x, Wq, Wk, Wv, Wp):
    bf16 = mybir.dt.np(BF16)
    x = np.asarray(x, dtype=np.float32).astype(bf16)
    wqt = np.ascontiguousarray(np.asarray(Wq, np.float32).T.astype(bf16))
    wkt = np.ascontiguousarray(np.asarray(Wk, np.float32).T.astype(bf16))
    wvt = np.ascontiguousarray(np.asarray(Wv, np.float32).T.astype(bf16))
    wpt = np.ascontiguousarray(np.asarray(Wp, np.float32).T)
    tri1 = (np.arange(KB)[:, None] <= np.arange(KB)[None, :]).astype(np.float32)
    tri = np.concatenate([tri1, np.zeros((KB, KB), np.float32), tri1], axis=1)
    in_maps = []
    for c in range(N_CORES):
        b, hg = c // 4, c % 4
        in_maps.append({
            "xt": np.ascontiguousarray(x[b].T),
            "wqt": np.ascontiguousarray(wqt[:, hg * GE:(hg + 1) * GE]),
            "wkt": np.ascontiguousarray(wkt[:, hg * GE:(hg + 1) * GE]),
            "wvt": np.ascontiguousarray(wvt[:, hg * GE:(hg + 1) * GE]),
            "wpt": np.ascontiguousarray(wpt[hg * GE:(hg + 1) * GE, :]),
            "tri": tri,
            "ones": np.ones((128, (T // KB) * GH), np.float32),
        })
    return in_maps


def run_cores(x, Wq, Wk, Wv, Wp, bp, **spmd_kwargs):
    """Run the 8-core program; returns (y_full, BassKernelResults)."""
    nc = _get_program()
    in_maps = _make_in_maps(x, Wq, Wk, Wv, Wp)
    res = run_bass_kernel_spmd(nc, in_maps, list(range(N_CORES)), **spmd_kwargs)
    parts = [np.asarray(res.results[c]["y"]).astype(np.float32) for c in range(N_CORES)]
    y = np.empty((B, T, E), np.float32)
    for b in range(B):
        acc = parts[4 * b]
        for hg in range(1, 4):
            acc = acc + parts[4 * b + hg]
        y[b] = acc
    y += np.asarray(bp, np.float32)[None, None, :]
    return y, res


def kernel(x, Wq, Wk, Wv, Wp, bp):
    y, _ = run_cores(x, Wq, Wk, Wv, Wp, bp)
    return y


def bench(x, Wq, Wk, Wv, Wp, bp, iters=12):
    """Time repeated on-device executions of the compiled program.

    Returns (y_full, list_of_call_seconds). Builds the sharded jit once;
    inputs are device-resident; fresh donated zero outputs are staged
    outside the timed region each iteration.
    """
    import time

    import jax
    import numpy as np_
    from jax.experimental.shard_map import shard_map
    from jax.sharding import Mesh, NamedSharding, PartitionSpec

    from concourse import bass2jax, mybir as mb

    nc = _get_program()
    in_maps = _make_in_maps(x, Wq, Wk, Wv, Wp)
    n_cores = N_CORES
    bass2jax.install_neuronx_cc_hook()

    partition_name = nc.partition_id_tensor.name if nc.partition_id_tensor else None
    in_names, out_names, out_avals, zero_outs = [], [], [], []
    for alloc in nc.m.functions[0].allocations:
        if not isinstance(alloc, mb.MemoryLocationSet):
            continue
        name = alloc.memorylocations[0].name
        if alloc.kind == "ExternalInput":
            if name != partition_name:
                in_names.append(name)
        elif alloc.kind == "ExternalOutput":
            out_names.append(name)
            shape = tuple(alloc.tensor_shape)
            dtype = mb.dt.np(alloc.dtype)
            out_avals.append(jax.core.ShapedArray(shape, dtype))
            zero_outs.append(np_.zeros(shape, dtype))
    n_params = len(in_names)
    all_in_names = in_names + out_names
    if partition_name is not None:
        all_in_names = all_in_names + [partition_name]

    def _body(*args):
        operands = list(args)
        if partition_name is not None:
            operands.append(bass2jax.partition_id_tensor())
        outs = bass2jax._bass_exec_p.bind(
            *operands,
            out_avals=tuple(out_avals),
            in_names=tuple(all_in_names),
            out_names=tuple(out_names),
            lowering_input_output_aliases=(),
            sim_require_finite=True,
            sim_require_nnan=True,
            nc=nc,
        )
        return tuple(outs)

    devices = jax.devices()[:n_cores]
    mesh = Mesh(np_.asarray(devices), ("core",))
    donate = tuple(range(n_params, n_params + len(out_names)))
    sharded = jax.jit(
        shard_map(_body, mesh=mesh,
                  in_specs=(PartitionSpec("core"),) * (n_params + len(out_names)),
                  out_specs=(PartitionSpec("core"),) * len(out_names),
                  check_rep=False),
        donate_argnums=donate, keep_unused=True,
    )
    sh = NamedSharding(mesh, PartitionSpec("core"))
    concat_in = [
        jax.device_put(
            np_.concatenate([np_.asarray(in_maps[c][nm]) for c in range(n_cores)], axis=0), sh)
        for nm in in_names
    ]
    zeros_np = [np_.zeros((n_cores * z.shape[0], *z.shape[1:]), z.dtype) for z in zero_outs]

    times = []
    out_arrs = None
    for it in range(iters):
        dz = [jax.device_put(z, sh) for z in zeros_np]
        jax.block_until_ready(dz)
        t0 = time.perf_counter()
        out_arrs = sharded(*concat_in, *dz)
        jax.block_until_ready(out_arrs)
        times.append(time.perf_counter() - t0)

    parts = [
        np_.asarray(out_arrs[i]).reshape(n_cores, *out_avals[i].shape)
        for i, nm in enumerate(out_names)
    ]
    yi = out_names.index("y")
    y = np_.empty((B, T, E), np_.float32)
    for b in range(B):
        acc = parts[yi][4 * b].astype(np_.float32)
        for hg in range(1, 4):
            acc = acc + parts[yi][4 * b + hg]
        y[b] = acc
    y += np_.asarray(bp, np_.float32)[None, None, :]
    return y, times


# revision 57
# speedup vs baseline: 1.2763x; 1.0062x over previous
"""Multi-head causal attention (B=2, T=2048, E=1024, H=16, D=64) on 8 trn2 cores.

Sharding: core c -> batch b = c // 4, head-group hg = c % 4 (4 heads each).
Per-core: QKV projections for its 4 heads, causal flash attention in
transposed-score layout (S^T[k,q]; softmax denominator folded into a
ones-augmented V matmul), row-parallel output projection producing a partial
[T, E] output. Host sums the 4 partials per batch and adds the bias.

Emission is software-pipelined: the attention stream for q-block qb
(S -> exp -> O, with S/exp skewed one unit ahead of O so the PE never
waits on the Activation engine) is interleaved with the projection
matmuls for t-chunk qb+1 and the output-projection matmuls of earlier
q-blocks, keeping the PE queue saturated while exp runs on Act.
"""
import sys
from collections import deque
from contextlib import ExitStack

sys.path.insert(0, "/opt/trn_rl_repo")

import numpy as np

import concourse.bass as bass
import concourse.tile as tile
from concourse import bacc, mybir
from concourse.bass_utils import run_bass_kernel_spmd

F32 = mybir.dt.float32
F32R = mybir.dt.float32r
BF16 = mybir.dt.bfloat16
EXP = mybir.ActivationFunctionType.Exp
COPY = mybir.ActivationFunctionType.Copy

B, T, E, H = 2, 2048, 1024, 16
D = E // H              # 64
N_CORES = 8
GH = 4                  # heads per core
GE = GH * D             # 256 per-core projection width
SCALE = float(D) ** -0.5

TCH = 512               # projection t-chunk
NTCH = T // TCH         # 4
KC = 8                  # contraction chunks of 128 over E
QB = 512                # attention q-block
NQB = T // QB           # 4
KB = 128                # attention k-block


DEFAULT_OPTS = dict(
    s_bufs=2,            # S psum slots ([128,1024] = 2 banks each)
    o_bufs=2,
    mix_bufs=2,          # shared q/k/v/y psum slots ([128,512] = 1 bank each)
    p_bufs=10,
    x_bufs=2,            # [128, KC, TCH] x tiles (one per t-chunk)
    on_bufs=8,
    yst_bufs=4,
    l_bufs=6,
    norm_splits_last=2,
    recip_fast=False,
    y_halves=True,       # y DMA per 512-wide half vs full tile
    startup=(("wq", 0, 4), ("x", 0, 2), ("wq", 4, 8), ("x", 2, 4),
             ("wk", 0, 8), ("x", 4, 8), ("wv", 0, 4), ("wv", 4, 8)),
    y_cp_alt=True,       # tail yp->yt copies alternate Act/DVE
    tail_pair=False,     # last q-block y DMAs batched per row-tile pair
    x0_act_dge=True,     # dispatch startup x DMAs from the idle Act DGE queue
)


def build_program(opts=None):
    o = dict(DEFAULT_OPTS)
    if opts:
        o.update(opts)
    nc = bacc.Bacc("TRN2", target_bir_lowering=False, debug=False, num_devices=N_CORES)

    xt_d = nc.dram_tensor("xt", [E, T], BF16, kind="ExternalInput").ap()
    wqt_d = nc.dram_tensor("wqt", [E, GE], BF16, kind="ExternalInput").ap()
    wkt_d = nc.dram_tensor("wkt", [E, GE], BF16, kind="ExternalInput").ap()
    wvt_d = nc.dram_tensor("wvt", [E, GE], BF16, kind="ExternalInput").ap()
    wpt_d = nc.dram_tensor("wpt", [GE, E], F32, kind="ExternalInput").ap()
    tri_d = nc.dram_tensor("tri", [KB, 3 * KB], F32, kind="ExternalInput").ap()
    ones_d = nc.dram_tensor("ones", [128, (T // KB) * GH], F32, kind="ExternalInput").ap()
    y_d = nc.dram_tensor("y", [T, E], BF16, kind="ExternalOutput").ap()

    with tile.TileContext(nc) as tc:
        with tc.tile_pool(name="weights", bufs=1) as wpool, \
             tc.tile_pool(name="qk", bufs=1) as qkpool, \
             tc.tile_pool(name="vsb", bufs=1) as vpool, \
             tc.tile_pool(name="xin", bufs=o["x_bufs"]) as xpool, \
             tc.tile_pool(name="ptile", bufs=o["p_bufs"]) as ppool, \
             tc.tile_pool(name="lbc", bufs=o["l_bufs"]) as lpool, \
             tc.tile_pool(name="onorm", bufs=o["on_bufs"]) as onpool, \
             tc.tile_pool(name="ystage", bufs=o["yst_bufs"]) as ypool, \
             tc.tile_pool(name="mix_ps", bufs=o["mix_bufs"], space="PSUM") as mix_ps, \
             tc.tile_pool(name="s_ps", bufs=o["s_bufs"], space="PSUM") as s_ps, \
             tc.tile_pool(name="o_ps", bufs=o["o_bufs"], space="PSUM") as o_ps:

            wq_sb = wpool.tile([128, KC, GE], BF16)
            wk_sb = wpool.tile([128, KC, GE], BF16)
            wv_sb = wpool.tile([128, KC, GE], BF16)
            wp_sb = wpool.tile([128, 2, E], F32R)
            tri_sb = wpool.tile([KB, 3 * KB], F32R)

            qt_sb = qkpool.tile([128, 2, T], F32R)   # pair-stacked Q^T
            kt_sb = qkpool.tile([128, 2, T], F32R)   # pair-stacked K^T
            v_sb = vpool.tile([128, T // KB, GH * (D + 1)], F32R)

            def w_dma(w_sb, w_d, c0, c1):
                nc.sync.dma_start(
                    out=w_sb[:, c0:c1, :],
                    in_=w_d.rearrange("(c p) n -> p c n", p=128)[:, c0:c1, :])

            def start_proj(tch):
                """Emit x DMA(s) for t-chunk tch; return the emission generator."""
                xt = xpool.tile([128, KC, TCH], BF16, tag="xt", name="xt")
                ts0 = tch * TCH
                if tch == 0:
                    # startup: transfers interleaved in need-order so the
                    # first matmuls begin as early as possible
                    x_eng = nc.scalar if o["x0_act_dge"] else nc.sync
                    def x_chunk(c0, c1):
                        x_eng.dma_start(out=xt[:, c0:c1, :],
                                        in_=xt_d.rearrange("(c p) t -> p c t", p=128)[:, c0:c1, ts0:ts0 + TCH])
                    for kind, c0, c1 in o["startup"]:
                        if kind == "x":
                            x_chunk(c0, c1)
                        elif kind == "wq":
                            w_dma(wq_sb, wqt_d, c0, c1)
                        elif kind == "wk":
                            w_dma(wk_sb, wkt_d, c0, c1)
                        elif kind == "wv":
                            w_dma(wv_sb, wvt_d, c0, c1)
                    nc.sync.dma_start(out=tri_sb[:], in_=tri_d.bitcast(F32R))
                    nc.sync.dma_start(out=wp_sb[:], in_=wpt_d.bitcast(F32R).rearrange("(c p) n -> p c n", p=128))
                else:
                    nc.sync.dma_start(
                        out=xt[:],
                        in_=xt_d.rearrange("(c p) t -> p c t", p=128)[:, :, ts0:ts0 + TCH])
                return proj_gen(tch, xt)

            def proj_gen(tch, xt):
                ts0 = tch * TCH
                for pair in range(2):
                    psl = slice(pair * 128, (pair + 1) * 128)
                    qp = mix_ps.tile([128, TCH], F32, tag="mix", name="qp")
                    for kc in range(KC):
                        nc.tensor.matmul(qp[:], wq_sb[:, kc, psl], xt[:, kc, :],
                                         start=(kc == 0), stop=(kc == KC - 1))
                        if kc % 2 == 1 and kc < KC - 1:
                            yield
                    nc.vector.tensor_copy(out=qt_sb[:, pair, ts0:ts0 + TCH], in_=qp[:])
                    yield
                    kp = mix_ps.tile([128, TCH], F32, tag="mix", name="kp")
                    for kc in range(KC):
                        nc.tensor.matmul(kp[:], wk_sb[:, kc, psl], xt[:, kc, :],
                                         start=(kc == 0), stop=(kc == KC - 1))
                        if kc % 2 == 1 and kc < KC - 1:
                            yield
                    nc.vector.tensor_copy(out=kt_sb[:, pair, ts0:ts0 + TCH], in_=kp[:])
                    yield
                for tsub in range(TCH // KB):
                    vp = mix_ps.tile([128, TCH], F32, tag="mix", name="vp")
                    for kc in range(KC):
                        nc.tensor.matmul(vp[:, 0:GE], xt[:, kc, tsub * KB:(tsub + 1) * KB],
                                         wv_sb[:, kc, :],
                                         start=(kc == 0), stop=(kc == KC - 1))
                        if kc == 3:
                            yield
                    tb = tch * (TCH // KB) + tsub
                    nc.vector.tensor_copy(
                        out=v_sb[:, tb, :].rearrange("p (h c) -> p h c", c=D + 1)[:, :, 0:D],
                        in_=vp[:, 0:GE].rearrange("p (h c) -> p h c", c=D),
                    )
                    yield

            def slot(hb):
                return slice(hb * (D + 1), (hb + 1) * (D + 1))

            def normalize(o_p, onorm, h, splits=1, only=None):
                w = QB // splits
                for s in range(splits):
                    if only is not None and s not in only:
                        continue
                    qs = slice(s * w, (s + 1) * w)
                    strip = lpool.tile([1, w], F32, tag="strip")
                    if o["recip_fast"]:
                        nc.vector.reciprocal_approx_fast(out=strip[:], in_=o_p[D:D + 1, qs])
                    else:
                        nc.vector.reciprocal(out=strip[:], in_=o_p[D:D + 1, qs])
                    lb = lpool.tile([D, w], F32, tag="lb")
                    nc.gpsimd.partition_broadcast(lb[:], strip[:])
                    nc.vector.tensor_mul(onorm[h * D:(h + 1) * D, qs], o_p[0:D, qs], lb[:])

            def emit_S_exp_full(sp, pt, bsl, pair, q0, j2):
                for jj in range(2):
                    j = j2 + jj
                    nc.tensor.matmul(sp[:, jj * QB:(jj + 1) * QB],
                                     kt_sb[bsl, pair, j * KB:(j + 1) * KB],
                                     qt_sb[bsl, pair, q0:q0 + QB],
                                     start=True, stop=True)
                nc.scalar.activation(out=pt[:], in_=sp[:], func=EXP, scale=SCALE)

            def emit_S_exp_diag(sp, pt, bsl, pair, q0, j2, nfull):
                for jj in range(2):
                    j = j2 + jj
                    r = (j - nfull) * KB
                    w = min(r, 256)
                    c0 = jj * QB
                    nc.tensor.matmul(sp[:, c0 + w:c0 + QB],
                                     kt_sb[bsl, pair, j * KB:(j + 1) * KB],
                                     qt_sb[bsl, pair, q0 + w:q0 + QB],
                                     start=True, stop=True)
                    nc.scalar.activation(out=pt[:, c0 + w:c0 + QB],
                                         in_=sp[:, c0 + w:c0 + QB],
                                         func=EXP, scale=SCALE)
                    if r == 384:
                        nc.vector.tensor_mul(pt[:, c0 + 256:c0 + QB],
                                             pt[:, c0 + 256:c0 + QB],
                                             tri_sb[:, KB:3 * KB])
                    else:
                        nc.vector.tensor_mul(pt[:, c0 + r:c0 + r + KB],
                                             pt[:, c0 + r:c0 + r + KB],
                                             tri_sb[:, 0:KB])

            # ---- pipelined emission ----
            gen = start_proj(0)
            for _ in gen:       # projections for t-chunk 0 up front
                pass
            # ones column of the augmented V (softmax denominator)
            v_ones = v_sb.rearrange("p b (h c) -> p (b h) c", c=D + 1)[:, :, D:D + 1]
            ones_sb = wpool.tile([128, (T // KB) * GH], F32R)
            nc.sync.dma_start(out=ones_sb[:], in_=ones_d.bitcast(F32R))
            nc.vector.tensor_copy(out=v_ones,
                                  in_=ones_sb.rearrange("p (n o) -> p n o", o=1))
            gen = start_proj(1)
            gen_rem = 24        # yields per proj generator

            fillers = deque()
            pend = []

            for qb in range(NQB):
                q0 = qb * QB
                nk = (q0 + QB) // KB
                nfull = nk - 4
                onorms = [onpool.tile([128, QB], F32R, tag="onorm", name="onorm")
                          for i in range(2)]
                n_units = 4 * (nfull // 2 + 2)
                slot_i = 0
                for pair in range(2):
                    for h in range(2):
                        hb = pair * 2 + h
                        bsl = slice(h * D, h * D + D)
                        o_p = o_ps.tile([D + 1, QB], F32, tag="o", name="o_p")
                        units = [("full", j2) for j2 in range(0, nfull, 2)] + \
                                [("diag", j2) for j2 in range(nfull, nk, 2)]
                        for ui, (kind, j2) in enumerate(units):
                            last = ui == len(units) - 1
                            sp = s_ps.tile([128, 2 * QB], F32, tag="s", name="sp")
                            pt = ppool.tile([128, 2 * QB], F32R, tag="p", name="pt")
                            if kind == "full":
                                emit_S_exp_full(sp, pt, bsl, pair, q0, j2)

                                def Bc(pt=pt, o_p=o_p, hb=hb, j2=j2):
                                    for jj in range(2):
                                        j = j2 + jj
                                        nc.tensor.matmul(o_p[:], v_sb[:, j, slot(hb)],
                                                         pt[:, jj * QB:(jj + 1) * QB],
                                                         start=(j == 0), stop=False)
                            else:
                                emit_S_exp_diag(sp, pt, bsl, pair, q0, j2, nfull)

                                def Bc(pt=pt, o_p=o_p, hb=hb, j2=j2, nfull=nfull, nk=nk):
                                    for jj in range(2):
                                        j = j2 + jj
                                        r = (j - nfull) * KB
                                        w = min(r, 256)
                                        nc.tensor.matmul(o_p[:, w:QB], v_sb[:, j, slot(hb)],
                                                         pt[:, jj * QB + w:(jj + 1) * QB],
                                                         start=(j == 0), stop=(j == nk - 1))
                            if last:
                                def B2(Bc=Bc, o_p=o_p, onorm=onorms[pair], h=h, qb=qb):
                                    Bc()
                                    if qb == NQB - 1:
                                        # cols [0:256] were normalized a unit
                                        # early (see below); finish [256:512]
                                        normalize(o_p, onorm, h, splits=2, only=(1,))
                                    else:
                                        normalize(o_p, onorm, h)
                                nxt = B2
                            elif qb == NQB - 1 and kind == "diag":
                                # first diagonal pair: o_p[:, 0:256] is final
                                # (later diag blocks only write [256:512]), so
                                # its normalize can start one unit early
                                def B2a(Bc=Bc, o_p=o_p, onorm=onorms[pair], h=h):
                                    Bc()
                                    normalize(o_p, onorm, h, splits=2, only=(0,))
                                nxt = B2a
                            else:
                                nxt = Bc
                            # pump proj filler, flush the PREVIOUS unit's O
                            # work (one-unit skew hides the exp latency), then
                            # queue this unit's O for the next slot
                            if gen is not None:
                                k = max(1, -(-gen_rem // (n_units - slot_i)))
                                for _ in range(k):
                                    try:
                                        next(gen)
                                        gen_rem -= 1
                                    except StopIteration:
                                        gen = None
                                        break
                            while pend:
                                pend.pop(0)()
                            pend.append(nxt)
                            # spread deferred out-projection work over the
                            # remaining proj-less slots
                            if gen is None and fillers and \
                                    len(fillers) * 2 >= n_units - slot_i:
                                fillers.popleft()()
                            slot_i += 1
                # drain any remaining proj work for the next t-chunk
                if gen is not None:
                    for _ in gen:
                        pass
                    gen = None
                if qb + 2 <= NTCH - 1:
                    gen = start_proj(qb + 2)
                    gen_rem = 24

                # out-projection for this q-block, deferred as filler work
                yt2_box = [None]
                for qt in range(QB // 128):
                    def F(qt=qt, q0=q0, onorms=onorms, alt=(qb == NQB - 1), yt2_box=yt2_box):
                        if alt and o["tail_pair"]:
                            # last q-block: pairs of row-tiles share one
                            # staging tile + a single batched DMA, shrinking
                            # the per-DMA dispatch tail
                            if qt % 2 == 0:
                                yt2_box[0] = ypool.tile([128, 2, E], BF16, tag="yt", name="yt")
                            yt = yt2_box[0][:, qt % 2, :]
                        else:
                            yt = ypool.tile([128, E], BF16, tag="yt", name="yt")
                        for nh in range(2):
                            if alt and (qt * 2 + nh) % 2 == 1:
                                yp = s_ps.tile([128, 512], F32, tag="s", name="yp")
                            else:
                                yp = mix_ps.tile([128, 512], F32, tag="mix", name="yp")
                            for pair in range(2):
                                nc.tensor.matmul(yp[:],
                                                 onorms[pair][:, qt * 128:(qt + 1) * 128],
                                                 wp_sb[:, pair, nh * 512:(nh + 1) * 512],
                                                 start=(pair == 0), stop=(pair == 1))
                            yh = yt[:, nh * 512:(nh + 1) * 512]
                            if o["y_cp_alt"] and alt and (qt * 2 + nh) % 2 == 0:
                                nc.scalar.activation(out=yh, in_=yp[:], func=COPY)
                            else:
                                nc.vector.tensor_copy(out=yh, in_=yp[:])
                            if not alt and o["y_halves"]:
                                nc.sync.dma_start(
                                    out=y_d[q0 + qt * 128:q0 + (qt + 1) * 128, nh * 512:(nh + 1) * 512],
                                    in_=yt[:, nh * 512:(nh + 1) * 512])
                        if alt and o["tail_pair"] and qt % 2 == 1:
                            nc.sync.dma_start(
                                out=y_d[q0 + (qt - 1) * 128:q0 + (qt + 1) * 128, :].rearrange(
                                    "(c p) n -> p c n", p=128),
                                in_=yt2_box[0][:])
                        elif alt and not o["tail_pair"]:
                            nc.sync.dma_start(out=y_d[q0 + qt * 128:q0 + (qt + 1) * 128, :], in_=yt[:])
                        elif not alt and not o["y_halves"]:
                            nc.sync.dma_start(out=y_d[q0 + qt * 128:q0 + (qt + 1) * 128, :], in_=yt[:])
                    fillers.append(F)
            while pend:
                pend.pop(0)()
            while fillers:
                fillers.popleft()()

    nc.compile()
    return nc


_NC = {}


def _get_program(opts=None):
    key = tuple(sorted((opts or {}).items()))
    if key not in _NC:
        _NC[key] = build_program(opts)
    return _NC[key]


def _make_in_maps(x, Wq, Wk, Wv, Wp):
    bf16 = mybir.dt.np(BF16)
    x = np.asarray(x, dtype=np.float32).astype(bf16)
    wqt = np.ascontiguousarray(np.asarray(Wq, np.float32).T.astype(bf16))
    wkt = np.ascontiguousarray(np.asarray(Wk, np.float32).T.astype(bf16))
    wvt = np.ascontiguousarray(np.asarray(Wv, np.float32).T.astype(bf16))
    wpt = np.ascontiguousarray(np.asarray(Wp, np.float32).T)
    tri1 = (np.arange(KB)[:, None] <= np.arange(KB)[None, :]).astype(np.float32)
    tri = np.concatenate([tri1, np.zeros((KB, KB), np.float32), tri1], axis=1)
    in_maps = []
    for c in range(N_CORES):
        b, hg = c // 4, c % 4
        in_maps.append({
            "xt": np.ascontiguousarray(x[b].T),
            "wqt": np.ascontiguousarray(wqt[:, hg * GE:(hg + 1) * GE]),
            "wkt": np.ascontiguousarray(wkt[:, hg * GE:(hg + 1) * GE]),
            "wvt": np.ascontiguousarray(wvt[:, hg * GE:(hg + 1) * GE]),
            "wpt": np.ascontiguousarray(wpt[hg * GE:(hg + 1) * GE, :]),
            "tri": tri,
            "ones": np.ones((128, (T // KB) * GH), np.float32),
        })
    return in_maps


def run_cores(x, Wq, Wk, Wv, Wp, bp, **spmd_kwargs):
    """Run the 8-core program; returns (y_full, BassKernelResults)."""
    nc = _get_program()
    in_maps = _make_in_maps(x, Wq, Wk, Wv, Wp)
    res = run_bass_kernel_spmd(nc, in_maps, list(range(N_CORES)), **spmd_kwargs)
    parts = [np.asarray(res.results[c]["y"]).astype(np.float32) for c in range(N_CORES)]
    y = np.empty((B, T, E), np.float32)
    for b in range(B):
        acc = parts[4 * b]
        for hg in range(1, 4):
            acc = acc + parts[4 * b + hg]
        y[b] = acc
    y += np.asarray(bp, np.float32)[None, None, :]
    return y, res


def kernel(x, Wq, Wk, Wv, Wp, bp):
    y, _ = run_cores(x, Wq, Wk, Wv, Wp, bp)
    return y


def bench(x, Wq, Wk, Wv, Wp, bp, iters=12):
    """Time repeated on-device executions of the compiled program.

    Returns (y_full, list_of_call_seconds). Builds the sharded jit once;
    inputs are device-resident; fresh donated zero outputs are staged
    outside the timed region each iteration.
    """
    import time

    import jax
    import numpy as np_
    from jax.experimental.shard_map import shard_map
    from jax.sharding import Mesh, NamedSharding, PartitionSpec

    from concourse import bass2jax, mybir as mb

    nc = _get_program()
    in_maps = _make_in_maps(x, Wq, Wk, Wv, Wp)
    n_cores = N_CORES
    bass2jax.install_neuronx_cc_hook()

    partition_name = nc.partition_id_tensor.name if nc.partition_id_tensor else None
    in_names, out_names, out_avals, zero_outs = [], [], [], []
    for alloc in nc.m.functions[0].allocations:
        if not isinstance(alloc, mb.MemoryLocationSet):
            continue
        name = alloc.memorylocations[0].name
        if alloc.kind == "ExternalInput":
            if name != partition_name:
                in_names.append(name)
        elif alloc.kind == "ExternalOutput":
            out_names.append(name)
            shape = tuple(alloc.tensor_shape)
            dtype = mb.dt.np(alloc.dtype)
            out_avals.append(jax.core.ShapedArray(shape, dtype))
            zero_outs.append(np_.zeros(shape, dtype))
    n_params = len(in_names)
    all_in_names = in_names + out_names
    if partition_name is not None:
        all_in_names = all_in_names + [partition_name]

    def _body(*args):
        operands = list(args)
        if partition_name is not None:
            operands.append(bass2jax.partition_id_tensor())
        outs = bass2jax._bass_exec_p.bind(
            *operands,
            out_avals=tuple(out_avals),
            in_names=tuple(all_in_names),
            out_names=tuple(out_names),
            lowering_input_output_aliases=(),
            sim_require_finite=True,
            sim_require_nnan=True,
            nc=nc,
        )
        return tuple(outs)

    devices = jax.devices()[:n_cores]
    mesh = Mesh(np_.asarray(devices), ("core",))
    donate = tuple(range(n_params, n_params + len(out_names)))
    sharded = jax.jit(
        shard_map(_body, mesh=mesh,
                  in_specs=(PartitionSpec("core"),) * (n_params + len(out_names)),
                  out_specs=(PartitionSpec("core"),) * len(out_names),
                  check_rep=False),
        donate_argnums=donate, keep_unused=True,
    )
    sh = NamedSharding(mesh, PartitionSpec("core"))
    concat_in = [
        jax.device_put(
            np_.concatenate([np_.asarray(in_maps[c][nm]) for c in range(n_cores)], axis=0), sh)
        for nm in in_names
    ]
    zeros_np = [np_.zeros((n_cores * z.shape[0], *z.shape[1:]), z.dtype) for z in zero_outs]

    times = []
    out_arrs = None
    for it in range(iters):
        dz = [jax.device_put(z, sh) for z in zeros_np]
        jax.block_until_ready(dz)
        t0 = time.perf_counter()
        out_arrs = sharded(*concat_in, *dz)
        jax.block_until_ready(out_arrs)
        times.append(time.perf_counter() - t0)

    parts = [
        np_.asarray(out_arrs[i]).reshape(n_cores, *out_avals[i].shape)
        for i, nm in enumerate(out_names)
    ]
    yi = out_names.index("y")
    y = np_.empty((B, T, E), np_.float32)
    for b in range(B):
        acc = parts[yi][4 * b].astype(np_.float32)
        for hg in range(1, 4):
            acc = acc + parts[yi][4 * b + hg]
        y[b] = acc
    y += np_.asarray(bp, np_.float32)[None, None, :]
    return y, times


# revision 59
# speedup vs baseline: 1.2858x; 1.0074x over previous
"""Multi-head causal attention (B=2, T=2048, E=1024, H=16, D=64) on 8 trn2 cores.

Sharding: core c -> batch b = c // 4, head-group hg = c % 4 (4 heads each).
Per-core: QKV projections for its 4 heads, causal flash attention in
transposed-score layout (S^T[k,q]; softmax denominator folded into a
ones-augmented V matmul), row-parallel output projection producing a partial
[T, E] output. Host sums the 4 partials per batch and adds the bias.

Emission is software-pipelined: the attention stream for q-block qb
(S -> exp -> O, with S/exp skewed one unit ahead of O so the PE never
waits on the Activation engine) is interleaved with the projection
matmuls for t-chunk qb+1 and the output-projection matmuls of earlier
q-blocks, keeping the PE queue saturated while exp runs on Act.
"""
import sys
from collections import deque
from contextlib import ExitStack

sys.path.insert(0, "/opt/trn_rl_repo")

import numpy as np

import concourse.bass as bass
import concourse.tile as tile
from concourse import bacc, mybir
from concourse.bass_utils import run_bass_kernel_spmd

F32 = mybir.dt.float32
F32R = mybir.dt.float32r
BF16 = mybir.dt.bfloat16
EXP = mybir.ActivationFunctionType.Exp
COPY = mybir.ActivationFunctionType.Copy

B, T, E, H = 2, 2048, 1024, 16
D = E // H              # 64
N_CORES = 8
GH = 4                  # heads per core
GE = GH * D             # 256 per-core projection width
SCALE = float(D) ** -0.5

TCH = 512               # projection t-chunk
NTCH = T // TCH         # 4
KC = 8                  # contraction chunks of 128 over E
QB = 512                # attention q-block
NQB = T // QB           # 4
KB = 128                # attention k-block


DEFAULT_OPTS = dict(
    s_bufs=2,            # S psum slots ([128,1024] = 2 banks each)
    o_bufs=2,
    mix_bufs=2,          # shared q/k/v/y psum slots ([128,512] = 1 bank each)
    p_bufs=10,
    x_bufs=2,            # [128, KC, TCH] x tiles (one per t-chunk)
    on_bufs=8,
    yst_bufs=4,
    l_bufs=6,
    norm_splits_last=2,
    recip_fast=False,
    y_halves=True,       # y DMA per 512-wide half vs full tile
    startup=(("wq", 0, 4), ("x", 0, 2), ("wq", 4, 8), ("x", 2, 4),
             ("wk", 0, 8), ("x", 4, 8), ("wv", 0, 4), ("wv", 4, 8)),
    y_cp_alt=True,       # tail yp->yt copies alternate Act/DVE
    tail_pair=False,     # last q-block y DMAs batched per row-tile pair
    x0_act_dge=False,    # dispatch startup x DMAs from the idle Act DGE queue
)


def build_program(opts=None):
    o = dict(DEFAULT_OPTS)
    if opts:
        o.update(opts)
    nc = bacc.Bacc("TRN2", target_bir_lowering=False, debug=False, num_devices=N_CORES)

    xt_d = nc.dram_tensor("xt", [E, T], BF16, kind="ExternalInput").ap()
    wqt_d = nc.dram_tensor("wqt", [E, GE], BF16, kind="ExternalInput").ap()
    wkt_d = nc.dram_tensor("wkt", [E, GE], BF16, kind="ExternalInput").ap()
    wvt_d = nc.dram_tensor("wvt", [E, GE], BF16, kind="ExternalInput").ap()
    wpt_d = nc.dram_tensor("wpt", [GE, E], F32, kind="ExternalInput").ap()
    tri_d = nc.dram_tensor("tri", [KB, KB], BF16, kind="ExternalInput").ap()
    ones_d = nc.dram_tensor("ones", [128, (T // KB) * GH], F32, kind="ExternalInput").ap()
    y_d = nc.dram_tensor("y", [T, E], BF16, kind="ExternalOutput").ap()

    with tile.TileContext(nc) as tc:
        with tc.tile_pool(name="weights", bufs=1) as wpool, \
             tc.tile_pool(name="qk", bufs=1) as qkpool, \
             tc.tile_pool(name="vsb", bufs=1) as vpool, \
             tc.tile_pool(name="xin", bufs=o["x_bufs"]) as xpool, \
             tc.tile_pool(name="ptile", bufs=o["p_bufs"]) as ppool, \
             tc.tile_pool(name="lbc", bufs=o["l_bufs"]) as lpool, \
             tc.tile_pool(name="onorm", bufs=o["on_bufs"]) as onpool, \
             tc.tile_pool(name="ystage", bufs=o["yst_bufs"]) as ypool, \
             tc.tile_pool(name="mix_ps", bufs=o["mix_bufs"], space="PSUM") as mix_ps, \
             tc.tile_pool(name="s_ps", bufs=o["s_bufs"], space="PSUM") as s_ps, \
             tc.tile_pool(name="o_ps", bufs=o["o_bufs"], space="PSUM") as o_ps:

            wq_sb = wpool.tile([128, KC, GE], BF16)
            wk_sb = wpool.tile([128, KC, GE], BF16)
            wv_sb = wpool.tile([128, KC, GE], BF16)
            wp_sb = wpool.tile([128, 2, E], F32R)
            tri_sb = wpool.tile([KB, KB], BF16)

            qt_sb = qkpool.tile([128, 2, T], BF16)   # pair-stacked Q^T
            kt_sb = qkpool.tile([128, 2, T], BF16)   # pair-stacked K^T
            v_sb = vpool.tile([128, T // KB, GH * (D + 1)], BF16)

            def w_dma(w_sb, w_d, c0, c1):
                nc.sync.dma_start(
                    out=w_sb[:, c0:c1, :],
                    in_=w_d.rearrange("(c p) n -> p c n", p=128)[:, c0:c1, :])

            def start_proj(tch):
                """Emit x DMA(s) for t-chunk tch; return the emission generator."""
                xt = xpool.tile([128, KC, TCH], BF16, tag="xt", name="xt")
                ts0 = tch * TCH
                if tch == 0:
                    # startup: transfers interleaved in need-order so the
                    # first matmuls begin as early as possible
                    x_eng = nc.scalar if o["x0_act_dge"] else nc.sync
                    def x_chunk(c0, c1):
                        x_eng.dma_start(out=xt[:, c0:c1, :],
                                        in_=xt_d.rearrange("(c p) t -> p c t", p=128)[:, c0:c1, ts0:ts0 + TCH])
                    for kind, c0, c1 in o["startup"]:
                        if kind == "x":
                            x_chunk(c0, c1)
                        elif kind == "wq":
                            w_dma(wq_sb, wqt_d, c0, c1)
                        elif kind == "wk":
                            w_dma(wk_sb, wkt_d, c0, c1)
                        elif kind == "wv":
                            w_dma(wv_sb, wvt_d, c0, c1)
                    nc.sync.dma_start(out=tri_sb[:], in_=tri_d)
                    nc.sync.dma_start(out=wp_sb[:], in_=wpt_d.bitcast(F32R).rearrange("(c p) n -> p c n", p=128))
                else:
                    nc.sync.dma_start(
                        out=xt[:],
                        in_=xt_d.rearrange("(c p) t -> p c t", p=128)[:, :, ts0:ts0 + TCH])
                return proj_gen(tch, xt)

            def proj_gen(tch, xt):
                ts0 = tch * TCH
                for pair in range(2):
                    psl = slice(pair * 128, (pair + 1) * 128)
                    qp = mix_ps.tile([128, TCH], F32, tag="mix", name="qp")
                    for kc in range(KC):
                        nc.tensor.matmul(qp[:], wq_sb[:, kc, psl], xt[:, kc, :],
                                         start=(kc == 0), stop=(kc == KC - 1))
                        if kc % 2 == 1 and kc < KC - 1:
                            yield
                    nc.vector.tensor_copy(out=qt_sb[:, pair, ts0:ts0 + TCH], in_=qp[:])
                    yield
                    kp = mix_ps.tile([128, TCH], F32, tag="mix", name="kp")
                    for kc in range(KC):
                        nc.tensor.matmul(kp[:], wk_sb[:, kc, psl], xt[:, kc, :],
                                         start=(kc == 0), stop=(kc == KC - 1))
                        if kc % 2 == 1 and kc < KC - 1:
                            yield
                    nc.vector.tensor_copy(out=kt_sb[:, pair, ts0:ts0 + TCH], in_=kp[:])
                    yield
                for tsub in range(TCH // KB):
                    vp = mix_ps.tile([128, TCH], F32, tag="mix", name="vp")
                    for kc in range(KC):
                        nc.tensor.matmul(vp[:, 0:GE], xt[:, kc, tsub * KB:(tsub + 1) * KB],
                                         wv_sb[:, kc, :],
                                         start=(kc == 0), stop=(kc == KC - 1))
                        if kc == 3:
                            yield
                    tb = tch * (TCH // KB) + tsub
                    nc.vector.tensor_copy(
                        out=v_sb[:, tb, :].rearrange("p (h c) -> p h c", c=D + 1)[:, :, 0:D],
                        in_=vp[:, 0:GE].rearrange("p (h c) -> p h c", c=D),
                    )
                    yield

            def slot(hb):
                return slice(hb * (D + 1), (hb + 1) * (D + 1))

            def normalize(o_p, onorm, h, splits=1, only=None):
                w = QB // splits
                for s in range(splits):
                    if only is not None and s not in only:
                        continue
                    qs = slice(s * w, (s + 1) * w)
                    strip = lpool.tile([1, w], F32, tag="strip")
                    if o["recip_fast"]:
                        nc.vector.reciprocal_approx_fast(out=strip[:], in_=o_p[D:D + 1, qs])
                    else:
                        nc.vector.reciprocal(out=strip[:], in_=o_p[D:D + 1, qs])
                    lb = lpool.tile([D, w], F32, tag="lb")
                    nc.gpsimd.partition_broadcast(lb[:], strip[:])
                    nc.vector.tensor_mul(onorm[h * D:(h + 1) * D, qs], o_p[0:D, qs], lb[:])

            def emit_S_exp_full(sp, pt, bsl, pair, q0, j2):
                for jj in range(2):
                    j = j2 + jj
                    nc.tensor.matmul(sp[:, jj * QB:(jj + 1) * QB],
                                     kt_sb[bsl, pair, j * KB:(j + 1) * KB],
                                     qt_sb[bsl, pair, q0:q0 + QB],
                                     start=True, stop=True)
                nc.scalar.activation(out=pt[:], in_=sp[:], func=EXP, scale=SCALE)

            def emit_S_exp_diag(sp, pt, bsl, pair, q0, j2, nfull):
                for jj in range(2):
                    j = j2 + jj
                    r = (j - nfull) * KB
                    c0 = jj * QB
                    nc.tensor.matmul(sp[:, c0 + r:c0 + QB],
                                     kt_sb[bsl, pair, j * KB:(j + 1) * KB],
                                     qt_sb[bsl, pair, q0 + r:q0 + QB],
                                     start=True, stop=True)
                    nc.scalar.activation(out=pt[:, c0 + r:c0 + QB],
                                         in_=sp[:, c0 + r:c0 + QB],
                                         func=EXP, scale=SCALE)
                    nc.vector.tensor_mul(pt[:, c0 + r:c0 + r + KB],
                                         pt[:, c0 + r:c0 + r + KB],
                                         tri_sb[:])

            # ---- pipelined emission ----
            gen = start_proj(0)
            for _ in gen:       # projections for t-chunk 0 up front
                pass
            # ones column of the augmented V (softmax denominator)
            v_ones = v_sb.rearrange("p b (h c) -> p (b h) c", c=D + 1)[:, :, D:D + 1]
            ones_sb = wpool.tile([128, (T // KB) * GH], F32)
            nc.sync.dma_start(out=ones_sb[:], in_=ones_d)
            nc.vector.tensor_copy(out=v_ones,
                                  in_=ones_sb.rearrange("p (n o) -> p n o", o=1))
            gen = start_proj(1)
            gen_rem = 24        # yields per proj generator

            fillers = deque()
            pend = []

            for qb in range(NQB):
                q0 = qb * QB
                nk = (q0 + QB) // KB
                nfull = nk - 4
                onorms = [onpool.tile([128, QB], F32R, tag="onorm", name="onorm")
                          for i in range(2)]
                n_units = 4 * (nfull // 2 + 2)
                slot_i = 0
                for pair in range(2):
                    for h in range(2):
                        hb = pair * 2 + h
                        bsl = slice(h * D, h * D + D)
                        o_p = o_ps.tile([D + 1, QB], F32, tag="o", name="o_p")
                        units = [("full", j2) for j2 in range(0, nfull, 2)] + \
                                [("diag", j2) for j2 in range(nfull, nk, 2)]
                        for ui, (kind, j2) in enumerate(units):
                            last = ui == len(units) - 1
                            sp = s_ps.tile([128, 2 * QB], F32, tag="s", name="sp")
                            pt = ppool.tile([128, 2 * QB], BF16, tag="p", name="pt")
                            if kind == "full":
                                emit_S_exp_full(sp, pt, bsl, pair, q0, j2)

                                def Bc(pt=pt, o_p=o_p, hb=hb, j2=j2):
                                    for jj in range(2):
                                        j = j2 + jj
                                        nc.tensor.matmul(o_p[:], v_sb[:, j, slot(hb)],
                                                         pt[:, jj * QB:(jj + 1) * QB],
                                                         start=(j == 0), stop=False)
                            else:
                                emit_S_exp_diag(sp, pt, bsl, pair, q0, j2, nfull)

                                def Bc(pt=pt, o_p=o_p, hb=hb, j2=j2, nfull=nfull, nk=nk):
                                    for jj in range(2):
                                        j = j2 + jj
                                        r = (j - nfull) * KB
                                        nc.tensor.matmul(o_p[:, r:QB], v_sb[:, j, slot(hb)],
                                                         pt[:, jj * QB + r:(jj + 1) * QB],
                                                         start=(j == 0), stop=(j == nk - 1))
                            if last:
                                def B2(Bc=Bc, o_p=o_p, onorm=onorms[pair], h=h, qb=qb):
                                    Bc()
                                    if qb == NQB - 1:
                                        # cols [0:256] were normalized a unit
                                        # early (see below); finish [256:512]
                                        normalize(o_p, onorm, h, splits=2, only=(1,))
                                    else:
                                        normalize(o_p, onorm, h)
                                nxt = B2
                            elif qb == NQB - 1 and kind == "diag":
                                # first diagonal pair: o_p[:, 0:256] is final
                                # (later diag blocks only write [256:512]), so
                                # its normalize can start one unit early
                                def B2a(Bc=Bc, o_p=o_p, onorm=onorms[pair], h=h):
                                    Bc()
                                    normalize(o_p, onorm, h, splits=2, only=(0,))
                                nxt = B2a
                            else:
                                nxt = Bc
                            # pump proj filler, flush the PREVIOUS unit's O
                            # work (one-unit skew hides the exp latency), then
                            # queue this unit's O for the next slot
                            if gen is not None:
                                k = max(1, -(-gen_rem // (n_units - slot_i)))
                                for _ in range(k):
                                    try:
                                        next(gen)
                                        gen_rem -= 1
                                    except StopIteration:
                                        gen = None
                                        break
                            while pend:
                                pend.pop(0)()
                            pend.append(nxt)
                            # spread deferred out-projection work over the
                            # remaining proj-less slots
                            if gen is None and fillers and \
                                    len(fillers) * 2 >= n_units - slot_i:
                                fillers.popleft()()
                            slot_i += 1
                # drain any remaining proj work for the next t-chunk
                if gen is not None:
                    for _ in gen:
                        pass
                    gen = None
                if qb + 2 <= NTCH - 1:
                    gen = start_proj(qb + 2)
                    gen_rem = 24

                # out-projection for this q-block, deferred as filler work
                yt2_box = [None]
                for qt in range(QB // 128):
                    def F(qt=qt, q0=q0, onorms=onorms, alt=(qb == NQB - 1), yt2_box=yt2_box):
                        if alt and o["tail_pair"]:
                            # last q-block: pairs of row-tiles share one
                            # staging tile + a single batched DMA, shrinking
                            # the per-DMA dispatch tail
                            if qt % 2 == 0:
                                yt2_box[0] = ypool.tile([128, 2, E], BF16, tag="yt", name="yt")
                            yt = yt2_box[0][:, qt % 2, :]
                        else:
                            yt = ypool.tile([128, E], BF16, tag="yt", name="yt")
                        for nh in range(2):
                            if alt and (qt * 2 + nh) % 2 == 1:
                                yp = s_ps.tile([128, 512], F32, tag="s", name="yp")
                            else:
                                yp = mix_ps.tile([128, 512], F32, tag="mix", name="yp")
                            for pair in range(2):
                                nc.tensor.matmul(yp[:],
                                                 onorms[pair][:, qt * 128:(qt + 1) * 128],
                                                 wp_sb[:, pair, nh * 512:(nh + 1) * 512],
                                                 start=(pair == 0), stop=(pair == 1))
                            yh = yt[:, nh * 512:(nh + 1) * 512]
                            if o["y_cp_alt"] and alt and (qt * 2 + nh) % 2 == 0:
                                nc.scalar.activation(out=yh, in_=yp[:], func=COPY)
                            else:
                                nc.vector.tensor_copy(out=yh, in_=yp[:])
                            if not alt and o["y_halves"]:
                                nc.sync.dma_start(
                                    out=y_d[q0 + qt * 128:q0 + (qt + 1) * 128, nh * 512:(nh + 1) * 512],
                                    in_=yt[:, nh * 512:(nh + 1) * 512])
                        if alt and o["tail_pair"] and qt % 2 == 1:
                            nc.sync.dma_start(
                                out=y_d[q0 + (qt - 1) * 128:q0 + (qt + 1) * 128, :].rearrange(
                                    "(c p) n -> p c n", p=128),
                                in_=yt2_box[0][:])
                        elif alt and not o["tail_pair"]:
                            nc.sync.dma_start(out=y_d[q0 + qt * 128:q0 + (qt + 1) * 128, :], in_=yt[:])
                        elif not alt and not o["y_halves"]:
                            nc.sync.dma_start(out=y_d[q0 + qt * 128:q0 + (qt + 1) * 128, :], in_=yt[:])
                    fillers.append(F)
            while pend:
                pend.pop(0)()
            while fillers:
                fillers.popleft()()

    nc.compile()
    return nc


_NC = {}


def _get_program(opts=None):
    key = tuple(sorted((opts or {}).items()))
    if key not in _NC:
        _NC[key] = build_program(opts)
    return _NC[key]


def _make_in_maps(x, Wq, Wk, Wv, Wp):
    bf16 = mybir.dt.np(BF16)
    x = np.asarray(x, dtype=np.float32).astype(bf16)
    wqt = np.ascontiguousarray(np.asarray(Wq, np.float32).T.astype(bf16))
    wkt = np.ascontiguousarray(np.asarray(Wk, np.float32).T.astype(bf16))
    wvt = np.ascontiguousarray(np.asarray(Wv, np.float32).T.astype(bf16))
    wpt = np.ascontiguousarray(np.asarray(Wp, np.float32).T)
    tri = (np.arange(KB)[:, None] <= np.arange(KB)[None, :]).astype(mybir.dt.np(BF16))
    in_maps = []
    for c in range(N_CORES):
        b, hg = c // 4, c % 4
        in_maps.append({
            "xt": np.ascontiguousarray(x[b].T),
            "wqt": np.ascontiguousarray(wqt[:, hg * GE:(hg + 1) * GE]),
            "wkt": np.ascontiguousarray(wkt[:, hg * GE:(hg + 1) * GE]),
            "wvt": np.ascontiguousarray(wvt[:, hg * GE:(hg + 1) * GE]),
            "wpt": np.ascontiguousarray(wpt[hg * GE:(hg + 1) * GE, :]),
            "tri": tri,
            "ones": np.ones((128, (T // KB) * GH), np.float32),
        })
    return in_maps


def run_cores(x, Wq, Wk, Wv, Wp, bp, **spmd_kwargs):
    """Run the 8-core program; returns (y_full, BassKernelResults)."""
    nc = _get_program()
    in_maps = _make_in_maps(x, Wq, Wk, Wv, Wp)
    res = run_bass_kernel_spmd(nc, in_maps, list(range(N_CORES)), **spmd_kwargs)
    parts = [np.asarray(res.results[c]["y"]).astype(np.float32) for c in range(N_CORES)]
    y = np.empty((B, T, E), np.float32)
    for b in range(B):
        acc = parts[4 * b]
        for hg in range(1, 4):
            acc = acc + parts[4 * b + hg]
        y[b] = acc
    y += np.asarray(bp, np.float32)[None, None, :]
    return y, res


def kernel(x, Wq, Wk, Wv, Wp, bp):
    y, _ = run_cores(x, Wq, Wk, Wv, Wp, bp)
    return y


def bench(x, Wq, Wk, Wv, Wp, bp, iters=12):
    """Time repeated on-device executions of the compiled program.

    Returns (y_full, list_of_call_seconds). Builds the sharded jit once;
    inputs are device-resident; fresh donated zero outputs are staged
    outside the timed region each iteration.
    """
    import time

    import jax
    import numpy as np_
    from jax.experimental.shard_map import shard_map
    from jax.sharding import Mesh, NamedSharding, PartitionSpec

    from concourse import bass2jax, mybir as mb

    nc = _get_program()
    in_maps = _make_in_maps(x, Wq, Wk, Wv, Wp)
    n_cores = N_CORES
    bass2jax.install_neuronx_cc_hook()

    partition_name = nc.partition_id_tensor.name if nc.partition_id_tensor else None
    in_names, out_names, out_avals, zero_outs = [], [], [], []
    for alloc in nc.m.functions[0].allocations:
        if not isinstance(alloc, mb.MemoryLocationSet):
            continue
        name = alloc.memorylocations[0].name
        if alloc.kind == "ExternalInput":
            if name != partition_name:
                in_names.append(name)
        elif alloc.kind == "ExternalOutput":
            out_names.append(name)
            shape = tuple(alloc.tensor_shape)
            dtype = mb.dt.np(alloc.dtype)
            out_avals.append(jax.core.ShapedArray(shape, dtype))
            zero_outs.append(np_.zeros(shape, dtype))
    n_params = len(in_names)
    all_in_names = in_names + out_names
    if partition_name is not None:
        all_in_names = all_in_names + [partition_name]

    def _body(*args):
        operands = list(args)
        if partition_name is not None:
            operands.append(bass2jax.partition_id_tensor())
        outs = bass2jax._bass_exec_p.bind(
            *operands,
            out_avals=tuple(out_avals),
            in_names=tuple(all_in_names),
            out_names=tuple(out_names),
            lowering_input_output_aliases=(),
            sim_require_finite=True,
            sim_require_nnan=True,
            nc=nc,
        )
        return tuple(outs)

    devices = jax.devices()[:n_cores]
    mesh = Mesh(np_.asarray(devices), ("core",))
    donate = tuple(range(n_params, n_params + len(out_names)))
    sharded = jax.jit(
        shard_map(_body, mesh=mesh,
                  in_specs=(PartitionSpec("core"),) * (n_params + len(out_names)),
                  out_specs=(PartitionSpec("core"),) * len(out_names),
                  check_rep=False),
        donate_argnums=donate, keep_unused=True,
    )
    sh = NamedSharding(mesh, PartitionSpec("core"))
    concat_in = [
        jax.device_put(
            np_.concatenate([np_.asarray(in_maps[c][nm]) for c in range(n_cores)], axis=0), sh)
        for nm in in_names
    ]
    zeros_np = [np_.zeros((n_cores * z.shape[0], *z.shape[1:]), z.dtype) for z in zero_outs]

    times = []
    out_arrs = None
    for it in range(iters):
        dz = [jax.device_put(z, sh) for z in zeros_np]
        jax.block_until_ready(dz)
        t0 = time.perf_counter()
        out_arrs = sharded(*concat_in, *dz)
        jax.block_until_ready(out_arrs)
        times.append(time.perf_counter() - t0)

    parts = [
        np_.asarray(out_arrs[i]).reshape(n_cores, *out_avals[i].shape)
        for i, nm in enumerate(out_names)
    ]
    yi = out_names.index("y")
    y = np_.empty((B, T, E), np_.float32)
    for b in range(B):
        acc = parts[yi][4 * b].astype(np_.float32)
        for hg in range(1, 4):
            acc = acc + parts[yi][4 * b + hg]
        y[b] = acc
    y += np_.asarray(bp, np_.float32)[None, None, :]
    return y, times
